# revision 11
# baseline (speedup 1.0000x reference)
"""GTN (Graph Transformer Network) message-passing on 8 trn2 NeuronCores.

Problem nn_GTN_17162689314910:
  A: [E=5, N=2048, N] f32, X: [2048, 256] f32, conv_w_*: [C=2, 5, 1, 1] f32,
  gcn_weight: [256, 64] f32 -> out [2048, 128] f32.

Distribution (channel x row split): core k -> channel c=k//4, row block
r=k%4 (512 rows). Per core (all heavy math on-device, bf16 matmuls):
  - combine a,b,a1 gtconv slices from its A rows (DVE, bf16)
  - AllGather b (4x 128-row chunks) within its 4-core group
  - layer0: H = a @ b  (PE, PSUM quarter-tiling), mask diag,
    PE-transpose -> H'T, colsum partial -> AllReduce -> scale = Hn1T
  - AllGather a1; layer1: H2 = Hn1 @ a1, mask diag -> H2'
  - colsum2 partial (PE ones-trick) -> ReduceScatter -> dinv2
  - readout partial H2'.T @ Xw -> ReduceScatter -> +Xw, *dinv2, relu
Host does only: softmax of the 10 conv scalars, sharding/transpose of
input slices, mask/identity constant construction, output concat.
"""
import os
import numpy as np
import ml_dtypes

import concourse.bass as bass
import concourse.tile as tile
from concourse import bacc, mybir
from concourse.bass_utils import run_bass_kernel_spmd

F32 = mybir.dt.float32
BF16 = mybir.dt.bfloat16
NPBF = ml_dtypes.bfloat16

N, E, C, NB, P = 2048, 5, 2, 512, 128
W_IN, W_OUT = 256, 64
NJT = N // P           # 16 j tiles
NMI = NB // P          # 4 row tiles per core block
GROUPS = [[0, 1, 2, 3], [4, 5, 6, 7]]


def _register_ntff_hook():
    """Best-effort: make trace=True work under axon when the image's
    antenv lacks axon_hooks (degrades silently otherwise)."""
    try:
        import antenv.axon_hooks  # noqa: F401
        return
    except ImportError:
        pass
    try:
        import sys, types, antenv
        from trn_agent_boot.trn_boot import _ntff_profile_via_ctypes
        mod = types.ModuleType("antenv.axon_hooks")
        _h = [None]
        mod.set_axon_ntff_profile_hook = lambda h: _h.__setitem__(0, h)
        mod.get_axon_ntff_profile_hook = lambda: _h[0]
        sys.modules["antenv.axon_hooks"] = mod
        antenv.axon_hooks = mod
        mod.set_axon_ntff_profile_hook(
            _ntff_profile_via_ctypes("/opt/axon/libaxon_pjrt.so"))
    except Exception:
        pass


def _build():
    nc = bacc.Bacc("TRN2", target_bir_lowering=False, debug=False,
                   num_devices=8)

    a_blk = nc.dram_tensor("a_blk", [E, NB, N], F32, kind="ExternalInput").ap()
    xt_blk = nc.dram_tensor("xt_blk", [W_IN, NB], F32, kind="ExternalInput").ap()
    w_gcn = nc.dram_tensor("w_gcn", [W_IN, W_OUT], F32, kind="ExternalInput").ap()
    scal = nc.dram_tensor("scal", [P, 16], F32, kind="ExternalInput").ap()
    ident = nc.dram_tensor("ident", [P, P], BF16, kind="ExternalInput").ap()
    dmask = nc.dram_tensor("dmask", [NMI, P, N], BF16, kind="ExternalInput").ap()
    out_x = nc.dram_tensor("out_x", [NB, W_OUT], F32, kind="ExternalOutput").ap()

    with tile.TileContext(nc) as tc:
        # ---- persistent pools (keep CM refs; GC would release pools) ----
        cms = {}

        def open_pool(**kw):
            cm = tc.tile_pool(**kw)
            pool = cm.__enter__()
            cms[kw["name"]] = cm
            return pool

        def close_pool(name):
            cms.pop(name).__exit__(None, None, None)

        dram = open_pool(name="dram", bufs=1, space="DRAM")
        const = open_pool(name="const", bufs=1)
        ps_mm = open_pool(name="ps_mm", bufs=6, space="PSUM")
        ps_msc = open_pool(name="ps_msc", bufs=2, space="PSUM")
        p_agt = open_pool(name="p_agt", bufs=NJT)
        p_hnt = open_pool(name="p_hnt", bufs=NJT)
        p_h2n = open_pool(name="p_h2n", bufs=NMI)
        p_msk = open_pool(name="p_msk", bufs=NJT)
        p_sml = open_pool(name="p_sml", bufs=2)

        # dram comm buffers
        ag_b_in = dram.tile([NB, N], BF16, tag="agbi", name="agbi")
        ag_b_out = dram.tile([N, N], BF16, tag="agbo", name="agbo")
        ag_c_in = dram.tile([NB, N], BF16, tag="agci", name="agci")
        ag_c_out = dram.tile([N, N], BF16, tag="agco", name="agco")
        cs1_in = dram.tile([P, NJT], F32, tag="cs1i", name="cs1i")
        cs1_out = dram.tile([P, NJT], F32, tag="cs1o", name="cs1o")
        cs2_in = dram.tile([1, N], F32, tag="cs2i", name="cs2i")
        cs2_out = dram.tile([NMI, P, 1], F32, tag="cs2o", name="cs2o")
        ro_in = dram.tile([NJT, P, W_OUT], F32, tag="roi", name="roi")
        ro_out = dram.tile([NMI, P, W_OUT], F32, tag="roo", name="roo")

        # constants
        sc = const.tile([P, 16], F32, tag="sc", name="sc")
        nc.sync.dma_start(sc[:], scal[:])
        ones = const.tile([P, 1], BF16, tag="ones", name="ones")
        nc.vector.memset(ones[:], 1.0)
        msk = [[None] * 4 for _ in range(NMI)]
        for m in range(NMI):
            for nq in range(4):
                mk = p_msk.tile([P, 512], BF16, tag="msk", name="msk")
                nc.sync.dma_start(mk[:], dmask[m, :, nq * 512:(nq + 1) * 512])
                msk[m][nq] = mk

        # ---- Xw = X[rows] @ W  (f32, tiny) ------------------------------
        xtt = [const.tile([P, NB], F32, tag=f"xtt{d}", name=f"xtt{d}") for d in range(2)]
        wt = [const.tile([P, W_OUT], F32, tag=f"wt{d}", name=f"wt{d}") for d in range(2)]
        for d in range(2):
            nc.sync.dma_start(xtt[d][:], xt_blk[d * P:(d + 1) * P, :])
            nc.sync.dma_start(wt[d][:], w_gcn[d * P:(d + 1) * P, :])
        xw_f = [const.tile([P, W_OUT], F32, tag=f"xwf{m}", name=f"xwf{m}") for m in range(NMI)]
        xw_b = [const.tile([P, W_OUT], BF16, tag=f"xwb{m}", name=f"xwb{m}") for m in range(NMI)]
        for m in range(NMI):
            pxw = ps_msc.tile([P, W_OUT], F32, tag="msc", name="msc")
            for d in range(2):
                nc.tensor.matmul(pxw[:], xtt[d][:, m * P:(m + 1) * P], wt[d][:],
                                 start=(d == 0), stop=(d == 1))
            nc.scalar.copy(xw_f[m][:], pxw[:])
            nc.vector.tensor_copy(xw_b[m][:], pxw[:])

        # ---- prologue: stage A (cast to bf16), combines, AGs ------------
        p_anat = open_pool(name="p_anat", bufs=NMI)
        p_nat = open_pool(name="p_nat", bufs=2)
        astage = open_pool(name="astage", bufs=8)

        a_nat = []
        for mi in range(NMI):
            at = [astage.tile([P, N], BF16, tag="ast", name="ast") for _ in range(E)]
            for e in range(E):
                nc.gpsimd.dma_start(at[e][:], a_blk[e, mi * P:(mi + 1) * P, :])

            def combine(dst, col0):
                t0 = p_nat.tile([P, N], BF16, tag="ctmp", name="ctmp")
                nc.vector.tensor_scalar_mul(dst[:], at[0][:], sc[:, col0:col0 + 1])
                for e in range(1, E):
                    nc.vector.tensor_scalar_mul(t0[:], at[e][:],
                                                sc[:, col0 + e:col0 + e + 1])
                    nc.vector.tensor_add(dst[:], dst[:], t0[:])

            # b slice
            b_nat = p_nat.tile([P, N], BF16, tag="bnat", name="bnat")
            combine(b_nat, 5)
            nc.sync.dma_start(ag_b_in[mi * P:(mi + 1) * P, :], b_nat[:])
            # a slice (layer-0 lhsT source, kept for transpose)
            an = p_anat.tile([P, N], BF16, tag="anat", name="anat")
            combine(an, 0)
            a_nat.append(an)
            # a1 slice
            c_nat = p_nat.tile([P, N], BF16, tag="cnat", name="cnat")
            combine(c_nat, 10)
            nc.sync.dma_start(ag_c_in[mi * P:(mi + 1) * P, :], c_nat[:])

        nc.gpsimd.collective_compute(
            "AllGather", mybir.AluOpType.bypass, replica_groups=GROUPS,
            ins=[ag_b_in.opt()], outs=[ag_b_out.opt()])
        nc.gpsimd.collective_compute(
            "AllGather", mybir.AluOpType.bypass, replica_groups=GROUPS,
            ins=[ag_c_in.opt()], outs=[ag_c_out.opt()])

        close_pool("astage")
        close_pool("p_nat")

        # transpose a_nat -> a_gT tiles [128(j), 512(i)] bf16 via DMA xbar
        agt = [p_agt.tile([P, NB], BF16, tag="agt", name="agt") for _ in range(NJT)]
        for mi in range(NMI):
            for jt in range(NJT):
                nc.scalar.dma_start(agt[jt][:, mi * P:(mi + 1) * P],
                                    a_nat[mi][:, jt * P:(jt + 1) * P],
                                    transpose=True)

        close_pool("p_anat")

        # b_sb resident tiles, loaded as AG chunks land: jt = 4*rk + mi
        p_bsb = open_pool(name="p_bsb", bufs=NJT)
        b_sb = []
        for jt in range(NJT):
            t = p_bsb.tile([P, N], BF16, tag="bsb", name="bsb")
            nc.sync.dma_start(t[:], ag_b_out[jt * P:(jt + 1) * P, :])
            b_sb.append(t)
        jt_order = list(range(NJT))

        # ---- layer 0: H = a @ b, mask diag, transpose, colsum ----------
        hnat = [p_h2n.tile([P, N], BF16, tag="h2n", name="h2n") for _ in range(NMI)]
        cs1_sb = p_sml.tile([P, NJT], F32, tag="cs1s", name="cs1s")
        hnt = [p_hnt.tile([P, NB], BF16, tag="hnt", name="hnt") for _ in range(NJT)]

        def bmm_layer(lhsT, rhs_sb, out_nat, jseq):
            """out_nat[m][:, :] = sum_j lhsT[jt][:, m*128:...] .T @ rhs_sb[jt],
            masked by dmask; PSUM quarter-tiled [128, 512]."""
            for nq in range(4):
                pq = [ps_mm.tile([P, 512], F32, tag="hacc", name="hacc") for _ in range(NMI)]
                for pos, jt in enumerate(jseq):
                    for m in range(NMI):
                        nc.tensor.matmul(
                            pq[m][:],
                            lhsT[jt][:, m * P:(m + 1) * P],
                            rhs_sb[jt][:, nq * 512:(nq + 1) * 512],
                            start=(pos == 0), stop=(pos == NJT - 1))
                for m in range(NMI):
                    nc.vector.tensor_mul(
                        out_nat[m][:, nq * 512:(nq + 1) * 512], pq[m][:],
                        msk[m][nq][:])

        bmm_layer(agt, b_sb, hnat, jt_order)

        # transpose H' -> H'T tiles [128(j), 512(i)] via DMA xbar; colsum
        for jt in range(NJT):
            for mi in range(NMI):
                nc.scalar.dma_start(hnt[jt][:, mi * P:(mi + 1) * P],
                                    hnat[mi][:, jt * P:(jt + 1) * P],
                                    transpose=True)
            nc.vector.tensor_reduce(cs1_sb[:, jt:jt + 1], hnt[jt][:],
                                    mybir.AxisListType.X, mybir.AluOpType.add)

        nc.sync.dma_start(cs1_in[:], cs1_sb[:])
        nc.gpsimd.collective_compute(
            "AllReduce", mybir.AluOpType.add, replica_groups=GROUPS,
            ins=[cs1_in.opt()], outs=[cs1_out.opt()])
        deg1 = p_sml.tile([P, NJT], F32, tag="deg1", name="deg1")
        nc.sync.dma_start(deg1[:], cs1_out[:])
        dinv1 = p_sml.tile([P, NJT], F32, tag="dinv1", name="dinv1")
        nc.vector.reciprocal(dinv1[:], deg1[:])
        for jt in range(NJT):
            nc.vector.tensor_scalar_mul(hnt[jt][:], hnt[jt][:],
                                        dinv1[:, jt:jt + 1])

        # a1 resident tiles from AG chunks
        p_csb = open_pool(name="p_csb", bufs=NJT)
        c_sb = []
        for jt in range(NJT):
            t = p_csb.tile([P, N], BF16, tag="csb", name="csb")
            nc.sync.dma_start(t[:], ag_c_out[jt * P:(jt + 1) * P, :])
            c_sb.append(t)

        # ---- layer 1: H2 = Hn1 @ a1, mask -> H2' -----------------------
        h2n = hnat  # reuse the same natural tiles (layer-0 copies are dead)
        bmm_layer(hnt, c_sb, h2n, list(range(NJT)))

        # ---- colsum2 (PE ones-trick) -> RS -> dinv2 --------------------
        for js in range(4):
            pcs = ps_msc.tile([1, 512], F32, tag="msc", name="msc")
            for m in range(NMI):
                nc.tensor.matmul(pcs[:], ones[:],
                                 h2n[m][:, js * 512:(js + 1) * 512],
                                 start=(m == 0), stop=(m == NMI - 1))
            cst = p_sml.tile([1, 512], F32, tag="cs2s", name="cs2s")
            nc.scalar.copy(cst[:], pcs[:])
            nc.sync.dma_start(cs2_in[:, js * 512:(js + 1) * 512], cst[:])
        nc.gpsimd.collective_compute(
            "ReduceScatter", mybir.AluOpType.add, replica_groups=GROUPS,
            ins=[cs2_in.opt()], outs=[cs2_out.opt()])
        deg2 = p_sml.tile([P, NMI], F32, tag="deg2", name="deg2")
        for r2 in range(NMI):
            nc.sync.dma_start(deg2[:, r2:r2 + 1], cs2_out[r2])
        deg2p = p_sml.tile([P, NMI], F32, tag="deg2p", name="deg2p")
        nc.vector.tensor_scalar_add(deg2p[:], deg2[:], 1.0)
        dinv2 = p_sml.tile([P, NMI], F32, tag="dinv2", name="dinv2")
        nc.vector.reciprocal(dinv2[:], deg2p[:])

        # ---- readout partials: P[mt] = sum_kt H2'[kt][:,mt].T @ Xw[kt] --
        for mt in range(NJT):
            pro = ps_msc.tile([P, W_OUT], F32, tag="msc", name="msc")
            for kt in range(NMI):
                nc.tensor.matmul(pro[:], h2n[kt][:, mt * P:(mt + 1) * P],
                                 xw_b[kt][:], start=(kt == 0), stop=(kt == NMI - 1))
            rot = p_sml.tile([P, W_OUT], F32, tag="ros", name="ros")
            nc.scalar.copy(rot[:], pro[:])
            nc.sync.dma_start(ro_in[mt][:, :], rot[:])
        nc.gpsimd.collective_compute(
            "ReduceScatter", mybir.AluOpType.add, replica_groups=GROUPS,
            ins=[ro_in.opt()], outs=[ro_out.opt()])

        # ---- epilogue: (RS + Xw) * dinv2, relu, store ------------------
        for s in range(NMI):
            t = p_sml.tile([P, W_OUT], F32, tag="ep0", name="ep0")
            nc.sync.dma_start(t[:], ro_out[s])
            t1 = p_sml.tile([P, W_OUT], F32, tag="ep1", name="ep1")
            nc.vector.tensor_add(t1[:], t[:], xw_f[s][:])
            t2 = p_sml.tile([P, W_OUT], F32, tag="ep2", name="ep2")
            nc.vector.tensor_scalar(t2[:], t1[:], dinv2[:, s:s + 1], 0.0,
                                    mybir.AluOpType.mult, mybir.AluOpType.max)
            nc.sync.dma_start(out_x[s * P:(s + 1) * P, :], t2[:])

        for nm in reversed(list(cms)):
            close_pool(nm)

    nc.compile()
    return nc


_NC = None


def _softmax(w):
    w = np.asarray(w, np.float64)
    m = w.max(axis=1, keepdims=True)
    e = np.exp(w - m)
    return (e / e.sum(axis=1, keepdims=True)).astype(np.float32)


def kernel(A, X, conv_w_l0_1, conv_w_l0_2, conv_w_l1, gcn_weight):
    global _NC
    if _NC is None:
        _NC = _build()
    A = np.ascontiguousarray(np.asarray(A, np.float32))
    X = np.asarray(X, np.float32)
    W = np.ascontiguousarray(np.asarray(gcn_weight, np.float32))
    s_a = _softmax(np.asarray(conv_w_l0_1, np.float32)[:, :, 0, 0])
    s_b = _softmax(np.asarray(conv_w_l0_2, np.float32)[:, :, 0, 0])
    s_a1 = _softmax(np.asarray(conv_w_l1, np.float32)[:, :, 0, 0])

    ident = np.eye(P, dtype=NPBF)
    in_maps = []
    for k in range(8):
        c, r = k // 4, k % 4
        rows = slice(NB * r, NB * r + NB)
        scal = np.zeros((P, 16), np.float32)
        scal[:, 0:5] = s_a[c]
        scal[:, 5:10] = s_b[c]
        scal[:, 10:15] = s_a1[c]
        dmask = np.ones((NMI, P, N), NPBF)
        idx = np.arange(P)
        for mi in range(NMI):
            dmask[mi, idx, NB * r + P * mi + idx] = NPBF(0.0)
        in_maps.append({
            "a_blk": np.ascontiguousarray(A[:, rows, :]),
            "xt_blk": np.ascontiguousarray(X[rows, :].T),
            "w_gcn": W,
            "scal": scal,
            "ident": ident,
            "dmask": dmask,
        })

    trace = bool(os.environ.get("GTN_TRACE"))
    if trace:
        _register_ntff_hook()
    res = run_bass_kernel_spmd(_NC, in_maps, list(range(8)), trace=trace)
    kernel.last_results = res

    out = np.zeros((N, C * W_OUT), np.float32)
    for k in range(8):
        c, r = k // 4, k % 4
        out[NB * r:NB * r + NB, c * W_OUT:(c + 1) * W_OUT] = \
            res.results[k]["out_x"]
    return out


# revision 14
# speedup vs baseline: 1.6409x; 1.6409x over previous
"""GTN (Graph Transformer Network) message-passing on 8 trn2 NeuronCores.

Problem nn_GTN_17162689314910:
  A: [E=5, N=2048, N] f32, X: [2048, 256] f32, conv_w_*: [C=2, 5, 1, 1] f32,
  gcn_weight: [256, 64] f32 -> out [2048, 128] f32.

Distribution (channel x row split): core k -> channel c=k//4, row block
r=k%4 (512 rows). Per core (heavy math on-device; fp8 wire, f32 accum):
  - combine a,b,a1 gtconv slices from its A rows (DVE; outputs fp8)
  - AllGather b, a1 (fp8) within its 4-core group
  - layer0 computed directly in TRANSPOSED form:
      H'T[jt] = lhsT(b tiles).T @ rhs(a.T tiles), masked by transposed
      diag-mask -> no PE transposes of H at all.
  - colsum partial -> AllReduce -> Hn1T = 256*dinv1 * H'T  (fp8)
  - layer1: H2 = Hn1 @ a1 (fp8 x fp8, x256 scale), drained through the
    natural diag-mask whose values are 2^-8 (mask + descale in one mul)
  - colsum2 partial (PE ones-trick) -> ReduceScatter -> dinv2
  - readout partial H2'.T @ Xw -> ReduceScatter -> +Xw, *dinv2, relu
Host does only: softmax of the 10 conv scalars, sharding/transpose of
input slices, mask/identity constant construction, output concat.
"""
import os
import numpy as np
import ml_dtypes

import concourse.bass as bass
import concourse.tile as tile
from concourse import bacc, mybir
from concourse.bass_utils import run_bass_kernel_spmd

F32 = mybir.dt.float32
BF16 = mybir.dt.bfloat16
FP8 = mybir.dt.float8e4
NPBF = ml_dtypes.bfloat16
NPF8 = np.dtype(mybir.dt.np(mybir.dt.float8e4))
DESCALE = np.float32(2.0 ** -8)

N, E, C, NB, P = 2048, 5, 2, 512, 128
W_IN, W_OUT = 256, 64
NJT = N // P           # 16 j tiles
NMI = NB // P          # 4 row tiles per core block
GROUPS = [[0, 1, 2, 3], [4, 5, 6, 7]]


def _register_ntff_hook():
    try:
        import antenv.axon_hooks  # noqa: F401
        return
    except ImportError:
        pass
    try:
        import sys, types, antenv
        from trn_agent_boot.trn_boot import _ntff_profile_via_ctypes
        mod = types.ModuleType("antenv.axon_hooks")
        _h = [None]
        mod.set_axon_ntff_profile_hook = lambda h: _h.__setitem__(0, h)
        mod.get_axon_ntff_profile_hook = lambda: _h[0]
        sys.modules["antenv.axon_hooks"] = mod
        antenv.axon_hooks = mod
        mod.set_axon_ntff_profile_hook(
            _ntff_profile_via_ctypes("/opt/axon/libaxon_pjrt.so"))
    except Exception:
        pass


def _build():
    nc = bacc.Bacc("TRN2", target_bir_lowering=False, debug=False,
                   num_devices=8)

    a_blk = nc.dram_tensor("a_blk", [E, NB, N], F32, kind="ExternalInput").ap()
    xt_blk = nc.dram_tensor("xt_blk", [W_IN, NB], F32, kind="ExternalInput").ap()
    w_gcn = nc.dram_tensor("w_gcn", [W_IN, W_OUT], F32, kind="ExternalInput").ap()
    scal = nc.dram_tensor("scal", [P, 16], F32, kind="ExternalInput").ap()
    ident = nc.dram_tensor("ident", [P, P], BF16, kind="ExternalInput").ap()
    # natural diag mask, values {2^-8, 0} (mask + layer-1 descale in one mul)
    dmask = nc.dram_tensor("dmask", [NMI, P, N], BF16, kind="ExternalInput").ap()
    # transposed diag mask, values {1, 0}
    dmaskt = nc.dram_tensor("dmaskt", [NJT, P, NB], BF16,
                            kind="ExternalInput").ap()
    out_x = nc.dram_tensor("out_x", [NB, W_OUT], F32, kind="ExternalOutput").ap()

    with tile.TileContext(nc) as tc:
        cms = {}

        def open_pool(**kw):
            cm = tc.tile_pool(**kw)
            pool = cm.__enter__()
            cms[kw["name"]] = cm
            return pool

        def close_pool(name):
            cms.pop(name).__exit__(None, None, None)

        dram = open_pool(name="dram", bufs=1, space="DRAM")
        const = open_pool(name="const", bufs=1)
        ps_mm = open_pool(name="ps_mm", bufs=4, space="PSUM")
        ps_tr = open_pool(name="ps_tr", bufs=2, space="PSUM")
        ps_msc = open_pool(name="ps_msc", bufs=2, space="PSUM")
        p_agt = open_pool(name="p_agt", bufs=NJT)
        p_msk = open_pool(name="p_msk", bufs=NJT)
        p_mskt = open_pool(name="p_mskt", bufs=NJT)
        p_hnt = open_pool(name="p_hnt", bufs=NJT)
        p_hn1 = open_pool(name="p_hn1", bufs=NJT)
        p_h2n = open_pool(name="p_h2n", bufs=NMI)
        p_sml = open_pool(name="p_sml", bufs=2)

        # dram comm buffers (fp8 wire for the big gathers)
        ag_b_in = dram.tile([NB, N], FP8, tag="agbi", name="agbi")
        ag_b_out = dram.tile([N, N], FP8, tag="agbo", name="agbo")
        ag_c_in = dram.tile([NB, N], FP8, tag="agci", name="agci")
        ag_c_out = dram.tile([N, N], FP8, tag="agco", name="agco")
        cs1_in = dram.tile([P, NJT], F32, tag="cs1i", name="cs1i")
        cs1_out = dram.tile([P, NJT], F32, tag="cs1o", name="cs1o")
        cs2_in = dram.tile([1, N], F32, tag="cs2i", name="cs2i")
        cs2_out = dram.tile([NMI, P, 1], F32, tag="cs2o", name="cs2o")
        ro_in = dram.tile([NJT, P, W_OUT], F32, tag="roi", name="roi")
        ro_out = dram.tile([NMI, P, W_OUT], F32, tag="roo", name="roo")

        # constants
        sc = const.tile([P, 16], F32, tag="sc", name="sc")
        nc.sync.dma_start(sc[:], scal[:])
        idt = const.tile([P, P], BF16, tag="idt", name="idt")
        nc.sync.dma_start(idt[:], ident[:])
        ones = const.tile([P, 1], BF16, tag="ones", name="ones")
        nc.vector.memset(ones[:], 1.0)
        msk = [[None] * 4 for _ in range(NMI)]
        for m in range(NMI):
            for nq in range(4):
                mk = p_msk.tile([P, 512], BF16, tag="msk", name="msk")
                nc.sync.dma_start(mk[:], dmask[m, :, nq * 512:(nq + 1) * 512])
                msk[m][nq] = mk
        mskt = []
        for jt in range(NJT):
            mk = p_mskt.tile([P, NB], BF16, tag="mskt", name="mskt")
            nc.sync.dma_start(mk[:], dmaskt[jt])
            mskt.append(mk)

        # ---- Xw = X[rows] @ W  (f32, tiny) ------------------------------
        xtt = [const.tile([P, NB], F32, tag=f"xtt{d}", name=f"xtt{d}")
               for d in range(2)]
        wt = [const.tile([P, W_OUT], F32, tag=f"wt{d}", name=f"wt{d}")
              for d in range(2)]
        for d in range(2):
            nc.sync.dma_start(xtt[d][:], xt_blk[d * P:(d + 1) * P, :])
            nc.sync.dma_start(wt[d][:], w_gcn[d * P:(d + 1) * P, :])
        xw_f = [const.tile([P, W_OUT], F32, tag=f"xwf{m}", name=f"xwf{m}")
                for m in range(NMI)]
        xw_b = [const.tile([P, W_OUT], BF16, tag=f"xwb{m}", name=f"xwb{m}")
                for m in range(NMI)]
        for m in range(NMI):
            pxw = ps_msc.tile([P, W_OUT], F32, tag="msc", name="pxw")
            for d in range(2):
                nc.tensor.matmul(pxw[:], xtt[d][:, m * P:(m + 1) * P], wt[d][:],
                                 start=(d == 0), stop=(d == 1))
            nc.scalar.copy(xw_f[m][:], pxw[:])
            nc.vector.tensor_copy(xw_b[m][:], pxw[:])

        # ---- prologue: stage A (cast to bf16), combines (fp8 out), AGs --
        p_anat = open_pool(name="p_anat", bufs=NMI)
        p_nat = open_pool(name="p_nat", bufs=2)
        astage = open_pool(name="astage", bufs=8)

        a_nat = []
        for mi in range(NMI):
            at = [astage.tile([P, N], BF16, tag="ast", name="ast")
                  for _ in range(E)]
            for e in range(E):
                nc.gpsimd.dma_start(at[e][:], a_blk[e, mi * P:(mi + 1) * P, :])

            def combine(dst, col0):
                # tree: ((A0*s0 + A1*s1) + A2*s2 + A3*s3) + A4*s4 -> dst
                t0 = p_nat.tile([P, N], BF16, tag="ct0", name="ct0")
                t1 = p_nat.tile([P, N], BF16, tag="ct1", name="ct1")
                nc.vector.tensor_scalar_mul(t0[:], at[0][:], sc[:, col0:col0 + 1])
                nc.vector.tensor_scalar_mul(t1[:], at[1][:],
                                            sc[:, col0 + 1:col0 + 2])
                nc.vector.tensor_add(t0[:], t0[:], t1[:])
                nc.vector.tensor_scalar_mul(t1[:], at[2][:],
                                            sc[:, col0 + 2:col0 + 3])
                nc.vector.tensor_add(t0[:], t0[:], t1[:])
                nc.vector.tensor_scalar_mul(t1[:], at[3][:],
                                            sc[:, col0 + 3:col0 + 4])
                nc.vector.tensor_add(t0[:], t0[:], t1[:])
                nc.vector.tensor_scalar_mul(t1[:], at[4][:],
                                            sc[:, col0 + 4:col0 + 5])
                nc.vector.tensor_add(dst[:], t0[:], t1[:])

            b_nat = p_nat.tile([P, N], FP8, tag="bnat", name="bnat")
            combine(b_nat, 5)
            nc.sync.dma_start(ag_b_in[mi * P:(mi + 1) * P, :], b_nat[:])
            an = p_anat.tile([P, N], BF16, tag="anat", name="anat")
            combine(an, 0)
            a_nat.append(an)
            c_nat = p_nat.tile([P, N], FP8, tag="cnat", name="cnat")
            combine(c_nat, 10)
            nc.sync.dma_start(ag_c_in[mi * P:(mi + 1) * P, :], c_nat[:])

        nc.gpsimd.collective_compute(
            "AllGather", mybir.AluOpType.bypass, replica_groups=GROUPS,
            ins=[ag_b_in.opt()], outs=[ag_b_out.opt()])
        nc.gpsimd.collective_compute(
            "AllGather", mybir.AluOpType.bypass, replica_groups=GROUPS,
            ins=[ag_c_in.opt()], outs=[ag_c_out.opt()])

        close_pool("astage")
        close_pool("p_nat")

        # transpose a_nat -> a_gT tiles [128(j), 512(i)] fp8 (PE transpose)
        agt = [p_agt.tile([P, NB], FP8, tag="agt", name="agt")
               for _ in range(NJT)]
        for mi in range(NMI):
            for jt in range(NJT):
                pt = ps_tr.tile([P, P], BF16, tag="tp", name="tp")
                nc.tensor.transpose(pt[:], a_nat[mi][:, jt * P:(jt + 1) * P],
                                    idt[:])
                nc.scalar.copy(agt[jt][:, mi * P:(mi + 1) * P], pt[:])

        close_pool("p_anat")

        # b tiles resident (fp8)
        p_bsb = open_pool(name="p_bsb", bufs=NJT)
        b_sb = []
        for jt in range(NJT):
            t = p_bsb.tile([P, N], FP8, tag="bsb", name="bsb")
            nc.sync.dma_start(t[:], ag_b_out[jt * P:(jt + 1) * P, :])
            b_sb.append(t)

        # ---- layer 0 in transposed form --------------------------------
        # H'T[jt][p, i] = sum_k b[k, 128jt+p] * a[own_i, k], masked.
        # lhsT = b_sb[kt][:, jt*128:+128]  (K=128 rows of b, M=128 j-cols)
        # rhs  = agt[kt]                   (K=128, N=512 own rows)
        cs1_sb = p_sml.tile([P, NJT], F32, tag="cs1s", name="cs1s")
        hnt = []
        for jt in range(NJT):
            pq = ps_mm.tile([P, NB], F32, tag="hacc", name="hacc")
            for kt in range(NJT):
                nc.tensor.matmul(pq[:], b_sb[kt][:, jt * P:(jt + 1) * P],
                                 agt[kt][:], start=(kt == 0),
                                 stop=(kt == NJT - 1))
            ht = p_hnt.tile([P, NB], BF16, tag="hnt", name="hnt")
            nc.vector.tensor_mul(ht[:], pq[:], mskt[jt][:])
            nc.vector.tensor_reduce(cs1_sb[:, jt:jt + 1], ht[:],
                                    mybir.AxisListType.X, mybir.AluOpType.add)
            hnt.append(ht)

        nc.sync.dma_start(cs1_in[:], cs1_sb[:])
        nc.gpsimd.collective_compute(
            "AllReduce", mybir.AluOpType.add, replica_groups=GROUPS,
            ins=[cs1_in.opt()], outs=[cs1_out.opt()])
        deg1 = p_sml.tile([P, NJT], F32, tag="deg1", name="deg1")
        nc.sync.dma_start(deg1[:], cs1_out[:])
        dinv1 = p_sml.tile([P, NJT], F32, tag="dinv1", name="dinv1")
        nc.vector.reciprocal(dinv1[:], deg1[:])
        d256 = p_sml.tile([P, NJT], F32, tag="d256", name="d256")
        nc.vector.tensor_scalar_mul(d256[:], dinv1[:], 256.0)
        # Hn1T (fp8, x256 scaled)
        hn1 = []
        for jt in range(NJT):
            h8 = p_hn1.tile([P, NB], FP8, tag="hn1", name="hn1")
            nc.vector.tensor_scalar_mul(h8[:], hnt[jt][:], d256[:, jt:jt + 1])
            hn1.append(h8)

        # a1 tiles resident (fp8)
        p_csb = open_pool(name="p_csb", bufs=NJT)
        c_sb = []
        for jt in range(NJT):
            t = p_csb.tile([P, N], FP8, tag="csb", name="csb")
            nc.sync.dma_start(t[:], ag_c_out[jt * P:(jt + 1) * P, :])
            c_sb.append(t)

        # ---- layer 1: H2 = Hn1 @ a1 (fp8, x256), drain descales --------
        h2n = [p_h2n.tile([P, N], BF16, tag="h2n", name="h2n")
               for _ in range(NMI)]
        for nq in range(4):
            pq = [ps_mm.tile([P, 512], F32, tag="hacc", name="hacc1")
                  for _ in range(NMI)]
            for kt in range(NJT):
                for m in range(NMI):
                    nc.tensor.matmul(
                        pq[m][:], hn1[kt][:, m * P:(m + 1) * P],
                        c_sb[kt][:, nq * 512:(nq + 1) * 512],
                        start=(kt == 0), stop=(kt == NJT - 1))
            for m in range(NMI):
                nc.vector.tensor_mul(h2n[m][:, nq * 512:(nq + 1) * 512],
                                     pq[m][:], msk[m][nq][:])

        # ---- colsum2 (PE ones-trick) -> RS -> dinv2 --------------------
        for js in range(4):
            pcs = ps_msc.tile([1, 512], F32, tag="msc", name="pcs")
            for m in range(NMI):
                nc.tensor.matmul(pcs[:], ones[:],
                                 h2n[m][:, js * 512:(js + 1) * 512],
                                 start=(m == 0), stop=(m == NMI - 1))
            cst = p_sml.tile([1, 512], F32, tag="cs2s", name="cs2s")
            nc.scalar.copy(cst[:], pcs[:])
            nc.sync.dma_start(cs2_in[:, js * 512:(js + 1) * 512], cst[:])
        nc.gpsimd.collective_compute(
            "ReduceScatter", mybir.AluOpType.add, replica_groups=GROUPS,
            ins=[cs2_in.opt()], outs=[cs2_out.opt()])
        deg2 = p_sml.tile([P, NMI], F32, tag="deg2", name="deg2")
        for r2 in range(NMI):
            nc.sync.dma_start(deg2[:, r2:r2 + 1], cs2_out[r2])
        deg2p = p_sml.tile([P, NMI], F32, tag="deg2p", name="deg2p")
        nc.vector.tensor_scalar_add(deg2p[:], deg2[:], 1.0)
        dinv2 = p_sml.tile([P, NMI], F32, tag="dinv2", name="dinv2")
        nc.vector.reciprocal(dinv2[:], deg2p[:])

        # ---- readout partials: P[mt] = sum_kt H2'[kt][:,mt].T @ Xw[kt] --
        for mt in range(NJT):
            pro = ps_msc.tile([P, W_OUT], F32, tag="msc", name="pro")
            for kt in range(NMI):
                nc.tensor.matmul(pro[:], h2n[kt][:, mt * P:(mt + 1) * P],
                                 xw_b[kt][:], start=(kt == 0),
                                 stop=(kt == NMI - 1))
            rot = p_sml.tile([P, W_OUT], F32, tag="ros", name="ros")
            nc.scalar.copy(rot[:], pro[:])
            nc.sync.dma_start(ro_in[mt][:, :], rot[:])
        nc.gpsimd.collective_compute(
            "ReduceScatter", mybir.AluOpType.add, replica_groups=GROUPS,
            ins=[ro_in.opt()], outs=[ro_out.opt()])

        # ---- epilogue: (RS + Xw) * dinv2, relu, store ------------------
        for s in range(NMI):
            t = p_sml.tile([P, W_OUT], F32, tag="ep0", name="ep0")
            nc.sync.dma_start(t[:], ro_out[s])
            t1 = p_sml.tile([P, W_OUT], F32, tag="ep1", name="ep1")
            nc.vector.tensor_add(t1[:], t[:], xw_f[s][:])
            t2 = p_sml.tile([P, W_OUT], F32, tag="ep2", name="ep2")
            nc.vector.tensor_scalar(t2[:], t1[:], dinv2[:, s:s + 1], 0.0,
                                    mybir.AluOpType.mult, mybir.AluOpType.max)
            nc.sync.dma_start(out_x[s * P:(s + 1) * P, :], t2[:])

        for nm in reversed(list(cms)):
            close_pool(nm)

    nc.compile()
    return nc


_NC = None


def _softmax(w):
    w = np.asarray(w, np.float64)
    m = w.max(axis=1, keepdims=True)
    e = np.exp(w - m)
    return (e / e.sum(axis=1, keepdims=True)).astype(np.float32)


def make_in_maps(A, X, conv_w_l0_1, conv_w_l0_2, conv_w_l1, gcn_weight):
    A = np.ascontiguousarray(np.asarray(A, np.float32))
    X = np.asarray(X, np.float32)
    W = np.ascontiguousarray(np.asarray(gcn_weight, np.float32))
    s_a = _softmax(np.asarray(conv_w_l0_1, np.float32)[:, :, 0, 0])
    s_b = _softmax(np.asarray(conv_w_l0_2, np.float32)[:, :, 0, 0])
    s_a1 = _softmax(np.asarray(conv_w_l1, np.float32)[:, :, 0, 0])

    ident = np.eye(P).astype(NPBF)
    idx = np.arange(P)
    in_maps = []
    for k in range(8):
        c, r = k // 4, k % 4
        rows = slice(NB * r, NB * r + NB)
        scal = np.zeros((P, 16), np.float32)
        scal[:, 0:5] = s_a[c]
        scal[:, 5:10] = s_b[c]
        scal[:, 10:15] = s_a1[c]
        # natural mask: {2^-8, 0}, diag at col 512r + 128mi + p
        dmask = np.full((NMI, P, N), NPBF(DESCALE), NPBF)
        for mi in range(NMI):
            dmask[mi, idx, NB * r + P * mi + idx] = NPBF(0.0)
        # transposed mask: {1, 0}, tile jt row p (j=128jt+p), col q=j-512r
        dmaskt = np.ones((NJT, P, NB), NPBF)
        for jt in range(NJT):
            j = P * jt + idx
            q = j - NB * r
            sel = (q >= 0) & (q < NB)
            dmaskt[jt, idx[sel], q[sel]] = NPBF(0.0)
        in_maps.append({
            "a_blk": np.ascontiguousarray(A[:, rows, :]),
            "xt_blk": np.ascontiguousarray(X[rows, :].T),
            "w_gcn": W,
            "scal": scal,
            "ident": ident,
            "dmask": dmask,
            "dmaskt": dmaskt,
        })
    return in_maps


def kernel(A, X, conv_w_l0_1, conv_w_l0_2, conv_w_l1, gcn_weight):
    global _NC
    if _NC is None:
        _NC = _build()
    in_maps = make_in_maps(A, X, conv_w_l0_1, conv_w_l0_2, conv_w_l1,
                           gcn_weight)
    trace = bool(os.environ.get("GTN_TRACE"))
    if trace:
        _register_ntff_hook()
    res = run_bass_kernel_spmd(_NC, in_maps, list(range(8)), trace=trace)
    kernel.last_results = res

    out = np.zeros((N, C * W_OUT), np.float32)
    for k in range(8):
        c, r = k // 4, k % 4
        out[NB * r:NB * r + NB, c * W_OUT:(c + 1) * W_OUT] = \
            res.results[k]["out_x"]
    return out


# revision 16
# speedup vs baseline: 1.6440x; 1.0019x over previous
"""GTN (Graph Transformer Network) message-passing on 8 trn2 NeuronCores.

Problem nn_GTN_17162689314910:
  A: [E=5, N=2048, N] f32, X: [2048, 256] f32, conv_w_*: [C=2, 5, 1, 1] f32,
  gcn_weight: [256, 64] f32 -> out [2048, 128] f32.

Distribution (channel x row split): core k -> channel c=k//4, row block
r=k%4 (512 rows). Per core (heavy math on-device; fp8 wire, f32 accum):
  - combine a,b,a1 gtconv slices from its A rows (DVE; outputs fp8)
  - AllGather b, a1 (fp8) within its 4-core group
  - layer0 computed directly in TRANSPOSED form:
      H'T[jt] = lhsT(b tiles).T @ rhs(a.T tiles), masked by transposed
      diag-mask -> no PE transposes of H at all.
  - colsum partial -> AllReduce -> Hn1T = 256*dinv1 * H'T  (fp8)
  - layer1: H2 = Hn1 @ a1 (fp8 x fp8, x256 scale), drained through the
    natural diag-mask whose values are 2^-8 (mask + descale in one mul)
  - colsum2 partial (PE ones-trick) -> ReduceScatter -> dinv2
  - readout partial H2'.T @ Xw -> ReduceScatter -> +Xw, *dinv2, relu
Host does only: softmax of the 10 conv scalars, sharding/transpose of
input slices, mask/identity constant construction, output concat.
"""
import os
import numpy as np
import ml_dtypes

import concourse.bass as bass
import concourse.tile as tile
from concourse import bacc, mybir
from concourse.bass_utils import run_bass_kernel_spmd

F32 = mybir.dt.float32
BF16 = mybir.dt.bfloat16
FP8 = mybir.dt.float8e4
NPBF = ml_dtypes.bfloat16
NPF8 = np.dtype(mybir.dt.np(mybir.dt.float8e4))
DESCALE = np.float32(2.0 ** -8)

N, E, C, NB, P = 2048, 5, 2, 512, 128
W_IN, W_OUT = 256, 64
NJT = N // P           # 16 j tiles
NMI = NB // P          # 4 row tiles per core block
GROUPS = [[0, 1, 2, 3], [4, 5, 6, 7]]


def _register_ntff_hook():
    try:
        import antenv.axon_hooks  # noqa: F401
        return
    except ImportError:
        pass
    try:
        import sys, types, antenv
        from trn_agent_boot.trn_boot import _ntff_profile_via_ctypes
        mod = types.ModuleType("antenv.axon_hooks")
        _h = [None]
        mod.set_axon_ntff_profile_hook = lambda h: _h.__setitem__(0, h)
        mod.get_axon_ntff_profile_hook = lambda: _h[0]
        sys.modules["antenv.axon_hooks"] = mod
        antenv.axon_hooks = mod
        mod.set_axon_ntff_profile_hook(
            _ntff_profile_via_ctypes("/opt/axon/libaxon_pjrt.so"))
    except Exception:
        pass


def _build():
    nc = bacc.Bacc("TRN2", target_bir_lowering=False, debug=False,
                   num_devices=8)

    a_blk = nc.dram_tensor("a_blk", [E, NB, N], F32, kind="ExternalInput").ap()
    xt_blk = nc.dram_tensor("xt_blk", [W_IN, NB], F32, kind="ExternalInput").ap()
    w_gcn = nc.dram_tensor("w_gcn", [W_IN, W_OUT], F32, kind="ExternalInput").ap()
    scal = nc.dram_tensor("scal", [P, 16], F32, kind="ExternalInput").ap()
    ident = nc.dram_tensor("ident", [P, P], BF16, kind="ExternalInput").ap()
    # natural diag mask, values {2^-8, 0} (mask + layer-1 descale in one mul)
    dmask = nc.dram_tensor("dmask", [NMI, P, N], BF16, kind="ExternalInput").ap()
    # transposed diag mask, values {1, 0}
    dmaskt = nc.dram_tensor("dmaskt", [NJT, P, NB], BF16,
                            kind="ExternalInput").ap()
    out_x = nc.dram_tensor("out_x", [NB, W_OUT], F32, kind="ExternalOutput").ap()

    with tile.TileContext(nc) as tc:
        cms = {}

        def open_pool(**kw):
            cm = tc.tile_pool(**kw)
            pool = cm.__enter__()
            cms[kw["name"]] = cm
            return pool

        def close_pool(name):
            cms.pop(name).__exit__(None, None, None)

        dram = open_pool(name="dram", bufs=1, space="DRAM")
        const = open_pool(name="const", bufs=1)
        ps_mm = open_pool(name="ps_mm", bufs=4, space="PSUM")
        ps_tr = open_pool(name="ps_tr", bufs=2, space="PSUM")
        ps_msc = open_pool(name="ps_msc", bufs=2, space="PSUM")
        p_agt = open_pool(name="p_agt", bufs=NJT)
        p_msk = open_pool(name="p_msk", bufs=NJT)
        p_mskt = open_pool(name="p_mskt", bufs=NJT)
        p_hnt = open_pool(name="p_hnt", bufs=NJT)
        p_hn1 = open_pool(name="p_hn1", bufs=NJT)
        p_sml = open_pool(name="p_sml", bufs=2)

        # dram comm buffers (fp8 wire for the big gathers)
        ag_b_in = dram.tile([NB, N], FP8, tag="agbi", name="agbi")
        ag_b_out = dram.tile([N, N], FP8, tag="agbo", name="agbo")
        ag_c_in = dram.tile([NB, N], FP8, tag="agci", name="agci")
        ag_c_out = dram.tile([N, N], FP8, tag="agco", name="agco")
        cs1_in = dram.tile([P, NJT], F32, tag="cs1i", name="cs1i")
        cs1_out = dram.tile([P, NJT], F32, tag="cs1o", name="cs1o")
        cs2_in = dram.tile([1, N], F32, tag="cs2i", name="cs2i")
        cs2_out = dram.tile([NMI, P, 1], F32, tag="cs2o", name="cs2o")
        ro_in = dram.tile([NJT, P, W_OUT], F32, tag="roi", name="roi")
        ro_out = dram.tile([NMI, P, W_OUT], F32, tag="roo", name="roo")

        # constants
        sc = const.tile([P, 16], F32, tag="sc", name="sc")
        nc.sync.dma_start(sc[:], scal[:])
        idt = const.tile([P, P], BF16, tag="idt", name="idt")
        nc.sync.dma_start(idt[:], ident[:])
        ones = const.tile([P, 1], BF16, tag="ones", name="ones")
        nc.vector.memset(ones[:], 1.0)
        msk = [[None] * 4 for _ in range(NMI)]
        for m in range(NMI):
            for nq in range(4):
                mk = p_msk.tile([P, 512], BF16, tag="msk", name="msk")
                nc.sync.dma_start(mk[:], dmask[m, :, nq * 512:(nq + 1) * 512])
                msk[m][nq] = mk
        mskt = []
        for jt in range(NJT):
            mk = p_mskt.tile([P, NB], BF16, tag="mskt", name="mskt")
            nc.sync.dma_start(mk[:], dmaskt[jt])
            mskt.append(mk)

        # ---- Xw = X[rows] @ W  (f32, tiny) ------------------------------
        xtt = [const.tile([P, NB], F32, tag=f"xtt{d}", name=f"xtt{d}")
               for d in range(2)]
        wt = [const.tile([P, W_OUT], F32, tag=f"wt{d}", name=f"wt{d}")
              for d in range(2)]
        for d in range(2):
            nc.sync.dma_start(xtt[d][:], xt_blk[d * P:(d + 1) * P, :])
            nc.sync.dma_start(wt[d][:], w_gcn[d * P:(d + 1) * P, :])
        xw_f = [const.tile([P, W_OUT], F32, tag=f"xwf{m}", name=f"xwf{m}")
                for m in range(NMI)]
        xw_b = [const.tile([P, W_OUT], BF16, tag=f"xwb{m}", name=f"xwb{m}")
                for m in range(NMI)]
        for m in range(NMI):
            pxw = ps_msc.tile([P, W_OUT], F32, tag="msc", name="pxw")
            for d in range(2):
                nc.tensor.matmul(pxw[:], xtt[d][:, m * P:(m + 1) * P], wt[d][:],
                                 start=(d == 0), stop=(d == 1))
            nc.scalar.copy(xw_f[m][:], pxw[:])
            nc.vector.tensor_copy(xw_b[m][:], pxw[:])

        # ---- prologue: stage A (cast to bf16), combines, AGs ------------
        # b-combines first so AG(b) launches early and overlaps the rest.
        p_anat = open_pool(name="p_anat", bufs=NMI)
        p_nat = open_pool(name="p_nat", bufs=2)
        astage = open_pool(name="astage", bufs=E * NMI)

        at_all = []
        for mi in range(NMI):
            at = [astage.tile([P, N], FP8, tag="ast", name="ast")
                  for _ in range(E)]
            for e in range(E):
                nc.gpsimd.dma_start(at[e][:], a_blk[e, mi * P:(mi + 1) * P, :])
            at_all.append(at)

        def combine(at, dst, col0):
            # scales split DVE/ACT, adds split DVE/GPSIMD
            t0 = p_nat.tile([P, N], BF16, tag="ct0", name="ct0")
            t1 = p_nat.tile([P, N], BF16, tag="ct1", name="ct1")
            t2 = p_nat.tile([P, N], BF16, tag="ct2", name="ct2")
            t3 = p_nat.tile([P, N], BF16, tag="ct3", name="ct3")
            nc.vector.tensor_scalar_mul(t0[:], at[0][:], sc[:, col0:col0 + 1])
            nc.vector.tensor_scalar_mul(t1[:], at[1][:],
                                        sc[:, col0 + 1:col0 + 2])
            nc.scalar.mul(t2[:], at[2][:], sc[:, col0 + 2:col0 + 3])
            nc.scalar.mul(t3[:], at[3][:], sc[:, col0 + 3:col0 + 4])
            nc.vector.tensor_add(t0[:], t0[:], t1[:])
            nc.gpsimd.tensor_add(t2[:], t2[:], t3[:])
            nc.vector.tensor_scalar_mul(t1[:], at[4][:],
                                        sc[:, col0 + 4:col0 + 5])
            nc.vector.tensor_add(t0[:], t0[:], t2[:])
            nc.vector.tensor_add(dst[:], t0[:], t1[:])

        for mi in range(NMI):
            b_nat = p_nat.tile([P, N], FP8, tag="bnat", name="bnat")
            combine(at_all[mi], b_nat, 5)
            nc.sync.dma_start(ag_b_in[mi * P:(mi + 1) * P, :], b_nat[:])

        nc.gpsimd.collective_compute(
            "AllGather", mybir.AluOpType.bypass, replica_groups=GROUPS,
            ins=[ag_b_in.opt()], outs=[ag_b_out.opt()])

        a_nat = []
        for mi in range(NMI):
            an = p_anat.tile([P, N], BF16, tag="anat", name="anat")
            combine(at_all[mi], an, 0)
            a_nat.append(an)
            c_nat = p_nat.tile([P, N], FP8, tag="cnat", name="cnat")
            combine(at_all[mi], c_nat, 10)
            nc.sync.dma_start(ag_c_in[mi * P:(mi + 1) * P, :], c_nat[:])

        nc.gpsimd.collective_compute(
            "AllGather", mybir.AluOpType.bypass, replica_groups=GROUPS,
            ins=[ag_c_in.opt()], outs=[ag_c_out.opt()])

        close_pool("astage")
        close_pool("p_nat")

        # transpose a_nat -> a_gT tiles [128(j), 512(i)] fp8 (PE transpose)
        agt = [p_agt.tile([P, NB], FP8, tag="agt", name="agt")
               for _ in range(NJT)]
        for mi in range(NMI):
            for jt in range(NJT):
                pt = ps_tr.tile([P, P], BF16, tag="tp", name="tp")
                nc.tensor.transpose(pt[:], a_nat[mi][:, jt * P:(jt + 1) * P],
                                    idt[:])
                nc.scalar.copy(agt[jt][:, mi * P:(mi + 1) * P], pt[:])

        close_pool("p_anat")

        p_h2n = open_pool(name="p_h2n", bufs=NMI)
        # b tiles resident (fp8)
        p_bsb = open_pool(name="p_bsb", bufs=NJT)
        b_sb = []
        for jt in range(NJT):
            t = p_bsb.tile([P, N], FP8, tag="bsb", name="bsb")
            nc.sync.dma_start(t[:], ag_b_out[jt * P:(jt + 1) * P, :])
            b_sb.append(t)

        # ---- layer 0 in transposed form --------------------------------
        # H'T[jt][p, i] = sum_k b[k, 128jt+p] * a[own_i, k], masked.
        # lhsT = b_sb[kt][:, jt*128:+128]  (K=128 rows of b, M=128 j-cols)
        # rhs  = agt[kt]                   (K=128, N=512 own rows)
        cs1_sb = p_sml.tile([P, NJT], F32, tag="cs1s", name="cs1s")
        hnt = []
        for jt in range(NJT):
            pq = ps_mm.tile([P, NB], F32, tag="hacc", name="hacc")
            for kt in range(NJT):
                nc.tensor.matmul(pq[:], b_sb[kt][:, jt * P:(jt + 1) * P],
                                 agt[kt][:], start=(kt == 0),
                                 stop=(kt == NJT - 1))
            ht = p_hnt.tile([P, NB], BF16, tag="hnt", name="hnt")
            nc.vector.tensor_mul(ht[:], pq[:], mskt[jt][:])
            nc.vector.tensor_reduce(cs1_sb[:, jt:jt + 1], ht[:],
                                    mybir.AxisListType.X, mybir.AluOpType.add)
            hnt.append(ht)

        nc.sync.dma_start(cs1_in[:], cs1_sb[:])
        nc.gpsimd.collective_compute(
            "AllReduce", mybir.AluOpType.add, replica_groups=GROUPS,
            ins=[cs1_in.opt()], outs=[cs1_out.opt()])
        deg1 = p_sml.tile([P, NJT], F32, tag="deg1", name="deg1")
        nc.sync.dma_start(deg1[:], cs1_out[:])
        dinv1 = p_sml.tile([P, NJT], F32, tag="dinv1", name="dinv1")
        nc.vector.reciprocal(dinv1[:], deg1[:])
        d256 = p_sml.tile([P, NJT], F32, tag="d256", name="d256")
        nc.vector.tensor_scalar_mul(d256[:], dinv1[:], 256.0)
        # Hn1T (fp8, x256 scaled)
        hn1 = []
        for jt in range(NJT):
            h8 = p_hn1.tile([P, NB], FP8, tag="hn1", name="hn1")
            nc.vector.tensor_scalar_mul(h8[:], hnt[jt][:], d256[:, jt:jt + 1])
            hn1.append(h8)

        # a1 tiles resident (fp8)
        p_csb = open_pool(name="p_csb", bufs=NJT)
        c_sb = []
        for jt in range(NJT):
            t = p_csb.tile([P, N], FP8, tag="csb", name="csb")
            nc.sync.dma_start(t[:], ag_c_out[jt * P:(jt + 1) * P, :])
            c_sb.append(t)

        # ---- layer 1: H2 = Hn1 @ a1 (fp8, x256), drain descales --------
        h2n = [p_h2n.tile([P, N], BF16, tag="h2n", name="h2n")
               for _ in range(NMI)]
        for nq in range(4):
            pq = [ps_mm.tile([P, 512], F32, tag="hacc", name="hacc1")
                  for _ in range(NMI)]
            for kt in range(NJT):
                for m in range(NMI):
                    nc.tensor.matmul(
                        pq[m][:], hn1[kt][:, m * P:(m + 1) * P],
                        c_sb[kt][:, nq * 512:(nq + 1) * 512],
                        start=(kt == 0), stop=(kt == NJT - 1))
            for m in range(NMI):
                nc.vector.tensor_mul(h2n[m][:, nq * 512:(nq + 1) * 512],
                                     pq[m][:], msk[m][nq][:])

        # ---- colsum2 (PE ones-trick) -> RS -> dinv2 --------------------
        for js in range(4):
            pcs = ps_msc.tile([1, 512], F32, tag="msc", name="pcs")
            for m in range(NMI):
                nc.tensor.matmul(pcs[:], ones[:],
                                 h2n[m][:, js * 512:(js + 1) * 512],
                                 start=(m == 0), stop=(m == NMI - 1))
            cst = p_sml.tile([1, 512], F32, tag="cs2s", name="cs2s")
            nc.scalar.copy(cst[:], pcs[:])
            nc.sync.dma_start(cs2_in[:, js * 512:(js + 1) * 512], cst[:])
        nc.gpsimd.collective_compute(
            "ReduceScatter", mybir.AluOpType.add, replica_groups=GROUPS,
            ins=[cs2_in.opt()], outs=[cs2_out.opt()])
        deg2 = p_sml.tile([P, NMI], F32, tag="deg2", name="deg2")
        for r2 in range(NMI):
            nc.sync.dma_start(deg2[:, r2:r2 + 1], cs2_out[r2])
        deg2p = p_sml.tile([P, NMI], F32, tag="deg2p", name="deg2p")
        nc.vector.tensor_scalar_add(deg2p[:], deg2[:], 1.0)
        dinv2 = p_sml.tile([P, NMI], F32, tag="dinv2", name="dinv2")
        nc.vector.reciprocal(dinv2[:], deg2p[:])

        # ---- readout partials: P[mt] = sum_kt H2'[kt][:,mt].T @ Xw[kt] --
        for mt in range(NJT):
            pro = ps_msc.tile([P, W_OUT], F32, tag="msc", name="pro")
            for kt in range(NMI):
                nc.tensor.matmul(pro[:], h2n[kt][:, mt * P:(mt + 1) * P],
                                 xw_b[kt][:], start=(kt == 0),
                                 stop=(kt == NMI - 1))
            rot = p_sml.tile([P, W_OUT], F32, tag="ros", name="ros")
            nc.scalar.copy(rot[:], pro[:])
            nc.sync.dma_start(ro_in[mt][:, :], rot[:])
        nc.gpsimd.collective_compute(
            "ReduceScatter", mybir.AluOpType.add, replica_groups=GROUPS,
            ins=[ro_in.opt()], outs=[ro_out.opt()])

        # ---- epilogue: (RS + Xw) * dinv2, relu, store ------------------
        for s in range(NMI):
            t = p_sml.tile([P, W_OUT], F32, tag="ep0", name="ep0")
            nc.sync.dma_start(t[:], ro_out[s])
            t1 = p_sml.tile([P, W_OUT], F32, tag="ep1", name="ep1")
            nc.vector.tensor_add(t1[:], t[:], xw_f[s][:])
            t2 = p_sml.tile([P, W_OUT], F32, tag="ep2", name="ep2")
            nc.vector.tensor_scalar(t2[:], t1[:], dinv2[:, s:s + 1], 0.0,
                                    mybir.AluOpType.mult, mybir.AluOpType.max)
            nc.sync.dma_start(out_x[s * P:(s + 1) * P, :], t2[:])

        for nm in reversed(list(cms)):
            close_pool(nm)

    nc.compile()
    return nc


_NC = None


def _softmax(w):
    w = np.asarray(w, np.float64)
    m = w.max(axis=1, keepdims=True)
    e = np.exp(w - m)
    return (e / e.sum(axis=1, keepdims=True)).astype(np.float32)


def make_in_maps(A, X, conv_w_l0_1, conv_w_l0_2, conv_w_l1, gcn_weight):
    A = np.ascontiguousarray(np.asarray(A, np.float32))
    X = np.asarray(X, np.float32)
    W = np.ascontiguousarray(np.asarray(gcn_weight, np.float32))
    s_a = _softmax(np.asarray(conv_w_l0_1, np.float32)[:, :, 0, 0])
    s_b = _softmax(np.asarray(conv_w_l0_2, np.float32)[:, :, 0, 0])
    s_a1 = _softmax(np.asarray(conv_w_l1, np.float32)[:, :, 0, 0])

    ident = np.eye(P).astype(NPBF)
    idx = np.arange(P)
    in_maps = []
    for k in range(8):
        c, r = k // 4, k % 4
        rows = slice(NB * r, NB * r + NB)
        scal = np.zeros((P, 16), np.float32)
        scal[:, 0:5] = s_a[c]
        scal[:, 5:10] = s_b[c]
        scal[:, 10:15] = s_a1[c]
        # natural mask: {2^-8, 0}, diag at col 512r + 128mi + p
        dmask = np.full((NMI, P, N), NPBF(DESCALE), NPBF)
        for mi in range(NMI):
            dmask[mi, idx, NB * r + P * mi + idx] = NPBF(0.0)
        # transposed mask: {1, 0}, tile jt row p (j=128jt+p), col q=j-512r
        dmaskt = np.ones((NJT, P, NB), NPBF)
        for jt in range(NJT):
            j = P * jt + idx
            q = j - NB * r
            sel = (q >= 0) & (q < NB)
            dmaskt[jt, idx[sel], q[sel]] = NPBF(0.0)
        in_maps.append({
            "a_blk": np.ascontiguousarray(A[:, rows, :]),
            "xt_blk": np.ascontiguousarray(X[rows, :].T),
            "w_gcn": W,
            "scal": scal,
            "ident": ident,
            "dmask": dmask,
            "dmaskt": dmaskt,
        })
    return in_maps


def kernel(A, X, conv_w_l0_1, conv_w_l0_2, conv_w_l1, gcn_weight):
    global _NC
    if _NC is None:
        _NC = _build()
    in_maps = make_in_maps(A, X, conv_w_l0_1, conv_w_l0_2, conv_w_l1,
                           gcn_weight)
    trace = bool(os.environ.get("GTN_TRACE"))
    if trace:
        _register_ntff_hook()
    res = run_bass_kernel_spmd(_NC, in_maps, list(range(8)), trace=trace)
    kernel.last_results = res

    out = np.zeros((N, C * W_OUT), np.float32)
    for k in range(8):
        c, r = k // 4, k % 4
        out[NB * r:NB * r + NB, c * W_OUT:(c + 1) * W_OUT] = \
            res.results[k]["out_x"]
    return out


# revision 17
# speedup vs baseline: 1.6940x; 1.0304x over previous
"""GTN (Graph Transformer Network) message-passing on 8 trn2 NeuronCores.

Problem nn_GTN_17162689314910:
  A: [E=5, N=2048, N] f32, X: [2048, 256] f32, conv_w_*: [C=2, 5, 1, 1] f32,
  gcn_weight: [256, 64] f32 -> out [2048, 128] f32.

Distribution (channel x row split): core k -> channel c=k//4, row block
r=k%4 (512 rows). Per core (heavy math on-device; fp8 wire, f32 accum):
  - combine a,b,a1 gtconv slices from its A rows (DVE; outputs fp8)
  - AllGather b, a1 (fp8) within its 4-core group
  - layer0 computed directly in TRANSPOSED form:
      H'T[jt] = lhsT(b tiles).T @ rhs(a.T tiles), masked by transposed
      diag-mask -> no PE transposes of H at all.
  - colsum partial -> AllReduce -> Hn1T = 256*dinv1 * H'T  (fp8)
  - layer1: H2 = Hn1 @ a1 (fp8 x fp8, x256 scale), drained through the
    natural diag-mask whose values are 2^-8 (mask + descale in one mul)
  - colsum2 partial (PE ones-trick) -> ReduceScatter -> dinv2
  - readout partial H2'.T @ Xw -> ReduceScatter -> +Xw, *dinv2, relu
Host does only: softmax of the 10 conv scalars, sharding/transpose of
input slices, mask/identity constant construction, output concat.
"""
import os
import numpy as np
import ml_dtypes

import concourse.bass as bass
import concourse.tile as tile
from concourse import bacc, mybir
from concourse.bass_utils import run_bass_kernel_spmd

F32 = mybir.dt.float32
BF16 = mybir.dt.bfloat16
FP8 = mybir.dt.float8e4
NPBF = ml_dtypes.bfloat16
NPF8 = np.dtype(mybir.dt.np(mybir.dt.float8e4))
DESCALE = np.float32(2.0 ** -8)

N, E, C, NB, P = 2048, 5, 2, 512, 128
W_IN, W_OUT = 256, 64
NJT = N // P           # 16 j tiles
NMI = NB // P          # 4 row tiles per core block
GROUPS = [[0, 1, 2, 3], [4, 5, 6, 7]]


def _register_ntff_hook():
    try:
        import antenv.axon_hooks  # noqa: F401
        return
    except ImportError:
        pass
    try:
        import sys, types, antenv
        from trn_agent_boot.trn_boot import _ntff_profile_via_ctypes
        mod = types.ModuleType("antenv.axon_hooks")
        _h = [None]
        mod.set_axon_ntff_profile_hook = lambda h: _h.__setitem__(0, h)
        mod.get_axon_ntff_profile_hook = lambda: _h[0]
        sys.modules["antenv.axon_hooks"] = mod
        antenv.axon_hooks = mod
        mod.set_axon_ntff_profile_hook(
            _ntff_profile_via_ctypes("/opt/axon/libaxon_pjrt.so"))
    except Exception:
        pass


def _build():
    nc = bacc.Bacc("TRN2", target_bir_lowering=False, debug=False,
                   num_devices=8)

    a_blk = nc.dram_tensor("a_blk", [E, NB, N], F32, kind="ExternalInput").ap()
    xt_blk = nc.dram_tensor("xt_blk", [W_IN, NB], F32, kind="ExternalInput").ap()
    w_gcn = nc.dram_tensor("w_gcn", [W_IN, W_OUT], F32, kind="ExternalInput").ap()
    scal = nc.dram_tensor("scal", [P, 16], F32, kind="ExternalInput").ap()
    ident = nc.dram_tensor("ident", [P, P], BF16, kind="ExternalInput").ap()
    # natural diag mask, values {2^-8, 0} (mask + layer-1 descale in one mul)
    dmask = nc.dram_tensor("dmask", [NMI, P, N], BF16, kind="ExternalInput").ap()
    # transposed diag mask, values {1, 0}
    dmaskt = nc.dram_tensor("dmaskt", [NJT, P, NB], BF16,
                            kind="ExternalInput").ap()
    out_x = nc.dram_tensor("out_x", [NB, W_OUT], F32, kind="ExternalOutput").ap()

    with tile.TileContext(nc) as tc:
        cms = {}

        def open_pool(**kw):
            cm = tc.tile_pool(**kw)
            pool = cm.__enter__()
            cms[kw["name"]] = cm
            return pool

        def close_pool(name):
            cms.pop(name).__exit__(None, None, None)

        dram = open_pool(name="dram", bufs=1, space="DRAM")
        const = open_pool(name="const", bufs=1)
        ps_mm = open_pool(name="ps_mm", bufs=4, space="PSUM")
        ps_tr = open_pool(name="ps_tr", bufs=2, space="PSUM")
        ps_msc = open_pool(name="ps_msc", bufs=2, space="PSUM")
        p_agt = open_pool(name="p_agt", bufs=NJT)
        p_mskt = open_pool(name="p_mskt", bufs=NJT)
        p_hnt = open_pool(name="p_hnt", bufs=NJT)
        p_hn1 = open_pool(name="p_hn1", bufs=NJT)
        p_sml = open_pool(name="p_sml", bufs=2)

        # dram comm buffers (fp8 wire for the big gathers)
        ag_b_in = dram.tile([NB, N], FP8, tag="agbi", name="agbi")
        ag_b_out = dram.tile([N, N], FP8, tag="agbo", name="agbo")
        ag_c_in = dram.tile([NB, N], FP8, tag="agci", name="agci")
        ag_c_out = dram.tile([N, N], FP8, tag="agco", name="agco")
        cs1_in = dram.tile([P, NJT], F32, tag="cs1i", name="cs1i")
        cs1_out = dram.tile([P, NJT], F32, tag="cs1o", name="cs1o")
        cs2_in = dram.tile([1, N], F32, tag="cs2i", name="cs2i")
        cs2_out = dram.tile([NMI, P, 1], F32, tag="cs2o", name="cs2o")
        ro_in = dram.tile([NJT, P, W_OUT], F32, tag="roi", name="roi")
        ro_out = dram.tile([NMI, P, W_OUT], F32, tag="roo", name="roo")

        # constants
        sc = const.tile([P, 16], F32, tag="sc", name="sc")
        nc.sync.dma_start(sc[:], scal[:])
        idt = const.tile([P, P], BF16, tag="idt", name="idt")
        nc.sync.dma_start(idt[:], ident[:])
        ones = const.tile([P, 1], BF16, tag="ones", name="ones")
        nc.vector.memset(ones[:], 1.0)
        mskt = []
        for jt in range(NJT):
            mk = p_mskt.tile([P, NB], BF16, tag="mskt", name="mskt")
            nc.sync.dma_start(mk[:], dmaskt[jt])
            mskt.append(mk)

        # ---- Xw = X[rows] @ W  (f32, tiny) ------------------------------
        xtt = [const.tile([P, NB], F32, tag=f"xtt{d}", name=f"xtt{d}")
               for d in range(2)]
        wt = [const.tile([P, W_OUT], F32, tag=f"wt{d}", name=f"wt{d}")
              for d in range(2)]
        for d in range(2):
            nc.sync.dma_start(xtt[d][:], xt_blk[d * P:(d + 1) * P, :])
            nc.sync.dma_start(wt[d][:], w_gcn[d * P:(d + 1) * P, :])
        xw_f = [const.tile([P, W_OUT], F32, tag=f"xwf{m}", name=f"xwf{m}")
                for m in range(NMI)]
        xw_b = [const.tile([P, W_OUT], BF16, tag=f"xwb{m}", name=f"xwb{m}")
                for m in range(NMI)]
        for m in range(NMI):
            pxw = ps_msc.tile([P, W_OUT], F32, tag="msc", name="pxw")
            for d in range(2):
                nc.tensor.matmul(pxw[:], xtt[d][:, m * P:(m + 1) * P], wt[d][:],
                                 start=(d == 0), stop=(d == 1))
            nc.scalar.copy(xw_f[m][:], pxw[:])
            nc.vector.tensor_copy(xw_b[m][:], pxw[:])

        # ---- prologue: stage A (cast to bf16), combines, AGs ------------
        # b-combines first so AG(b) launches early and overlaps the rest.
        p_anat = open_pool(name="p_anat", bufs=NMI)
        p_nat = open_pool(name="p_nat", bufs=2)
        astage = open_pool(name="astage", bufs=E * NMI)

        at_all = []
        for mi in range(NMI):
            at = [astage.tile([P, N], BF16, tag="ast", name="ast")
                  for _ in range(E)]
            for e in range(E):
                nc.gpsimd.dma_start(at[e][:], a_blk[e, mi * P:(mi + 1) * P, :])
            at_all.append(at)

        def combine(at, dst, col0):
            # scales split DVE(3)/ACT(2); adds on DVE
            t0 = p_nat.tile([P, N], BF16, tag="ct0", name="ct0")
            t1 = p_nat.tile([P, N], BF16, tag="ct1", name="ct1")
            t2 = p_nat.tile([P, N], BF16, tag="ct2", name="ct2")
            t3 = p_nat.tile([P, N], BF16, tag="ct3", name="ct3")
            nc.vector.tensor_scalar_mul(t0[:], at[0][:], sc[:, col0:col0 + 1])
            nc.vector.tensor_scalar_mul(t1[:], at[1][:],
                                        sc[:, col0 + 1:col0 + 2])
            nc.scalar.mul(t2[:], at[2][:], sc[:, col0 + 2:col0 + 3])
            nc.scalar.mul(t3[:], at[3][:], sc[:, col0 + 3:col0 + 4])
            nc.vector.tensor_add(t0[:], t0[:], t1[:])
            nc.vector.tensor_scalar_mul(t1[:], at[4][:],
                                        sc[:, col0 + 4:col0 + 5])
            nc.vector.tensor_add(t2[:], t2[:], t3[:])
            nc.vector.tensor_add(t0[:], t0[:], t2[:])
            nc.vector.tensor_add(dst[:], t0[:], t1[:])

        for mi in range(NMI):
            b_nat = p_nat.tile([P, N], FP8, tag="bnat", name="bnat")
            combine(at_all[mi], b_nat, 5)
            nc.sync.dma_start(ag_b_in[mi * P:(mi + 1) * P, :], b_nat[:])

        nc.gpsimd.collective_compute(
            "AllGather", mybir.AluOpType.bypass, replica_groups=GROUPS,
            ins=[ag_b_in.opt()], outs=[ag_b_out.opt()])

        a_nat = []
        for mi in range(NMI):
            an = p_anat.tile([P, N], BF16, tag="anat", name="anat")
            combine(at_all[mi], an, 0)
            a_nat.append(an)
            c_nat = p_nat.tile([P, N], FP8, tag="cnat", name="cnat")
            combine(at_all[mi], c_nat, 10)
            nc.sync.dma_start(ag_c_in[mi * P:(mi + 1) * P, :], c_nat[:])

        nc.gpsimd.collective_compute(
            "AllGather", mybir.AluOpType.bypass, replica_groups=GROUPS,
            ins=[ag_c_in.opt()], outs=[ag_c_out.opt()])

        close_pool("astage")
        close_pool("p_nat")

        # transpose a_nat -> a_gT tiles [128(j), 512(i)] fp8 (PE transpose)
        agt = [p_agt.tile([P, NB], FP8, tag="agt", name="agt")
               for _ in range(NJT)]
        for mi in range(NMI):
            for jt in range(NJT):
                pt = ps_tr.tile([P, P], BF16, tag="tp", name="tp")
                nc.tensor.transpose(pt[:], a_nat[mi][:, jt * P:(jt + 1) * P],
                                    idt[:])
                nc.scalar.copy(agt[jt][:, mi * P:(mi + 1) * P], pt[:])

        close_pool("p_anat")

        p_msk = open_pool(name="p_msk", bufs=NJT)
        msk = [[None] * 4 for _ in range(NMI)]
        for m in range(NMI):
            for nq in range(4):
                mk = p_msk.tile([P, 512], BF16, tag="msk", name="msk")
                nc.sync.dma_start(mk[:], dmask[m, :, nq * 512:(nq + 1) * 512])
                msk[m][nq] = mk
        p_h2n = open_pool(name="p_h2n", bufs=NMI)
        # b tiles resident (fp8)
        p_bsb = open_pool(name="p_bsb", bufs=NJT)
        b_sb = []
        for jt in range(NJT):
            t = p_bsb.tile([P, N], FP8, tag="bsb", name="bsb")
            nc.sync.dma_start(t[:], ag_b_out[jt * P:(jt + 1) * P, :])
            b_sb.append(t)

        # ---- layer 0 in transposed form --------------------------------
        # H'T[jt][p, i] = sum_k b[k, 128jt+p] * a[own_i, k], masked.
        # lhsT = b_sb[kt][:, jt*128:+128]  (K=128 rows of b, M=128 j-cols)
        # rhs  = agt[kt]                   (K=128, N=512 own rows)
        cs1_sb = p_sml.tile([P, NJT], F32, tag="cs1s", name="cs1s")
        hnt = []
        for jt in range(NJT):
            pq = ps_mm.tile([P, NB], F32, tag="hacc", name="hacc")
            for kt in range(NJT):
                nc.tensor.matmul(pq[:], b_sb[kt][:, jt * P:(jt + 1) * P],
                                 agt[kt][:], start=(kt == 0),
                                 stop=(kt == NJT - 1))
            ht = p_hnt.tile([P, NB], BF16, tag="hnt", name="hnt")
            nc.vector.tensor_mul(ht[:], pq[:], mskt[jt][:])
            nc.vector.tensor_reduce(cs1_sb[:, jt:jt + 1], ht[:],
                                    mybir.AxisListType.X, mybir.AluOpType.add)
            hnt.append(ht)

        nc.sync.dma_start(cs1_in[:], cs1_sb[:])
        nc.gpsimd.collective_compute(
            "AllReduce", mybir.AluOpType.add, replica_groups=GROUPS,
            ins=[cs1_in.opt()], outs=[cs1_out.opt()])
        deg1 = p_sml.tile([P, NJT], F32, tag="deg1", name="deg1")
        nc.sync.dma_start(deg1[:], cs1_out[:])
        dinv1 = p_sml.tile([P, NJT], F32, tag="dinv1", name="dinv1")
        nc.vector.reciprocal(dinv1[:], deg1[:])
        d256 = p_sml.tile([P, NJT], F32, tag="d256", name="d256")
        nc.vector.tensor_scalar_mul(d256[:], dinv1[:], 256.0)
        # Hn1T (fp8, x256 scaled)
        hn1 = []
        for jt in range(NJT):
            h8 = p_hn1.tile([P, NB], FP8, tag="hn1", name="hn1")
            nc.scalar.mul(h8[:], hnt[jt][:], d256[:, jt:jt + 1])
            hn1.append(h8)

        # a1 tiles resident (fp8)
        p_csb = open_pool(name="p_csb", bufs=NJT)
        c_sb = []
        for jt in range(NJT):
            t = p_csb.tile([P, N], FP8, tag="csb", name="csb")
            nc.sync.dma_start(t[:], ag_c_out[jt * P:(jt + 1) * P, :])
            c_sb.append(t)

        # ---- layer 1: H2 = Hn1 @ a1 (fp8, x256), drain descales --------
        h2n = [p_h2n.tile([P, N], BF16, tag="h2n", name="h2n")
               for _ in range(NMI)]
        for nq in range(4):
            pq = [ps_mm.tile([P, 512], F32, tag="hacc", name="hacc1")
                  for _ in range(NMI)]
            for kt in range(NJT):
                for m in range(NMI):
                    nc.tensor.matmul(
                        pq[m][:], hn1[kt][:, m * P:(m + 1) * P],
                        c_sb[kt][:, nq * 512:(nq + 1) * 512],
                        start=(kt == 0), stop=(kt == NJT - 1))
            for m in range(NMI):
                nc.vector.tensor_mul(h2n[m][:, nq * 512:(nq + 1) * 512],
                                     pq[m][:], msk[m][nq][:])

        # ---- colsum2 (PE ones-trick) -> RS -> dinv2 --------------------
        for js in range(4):
            pcs = ps_msc.tile([1, 512], F32, tag="msc", name="pcs")
            for m in range(NMI):
                nc.tensor.matmul(pcs[:], ones[:],
                                 h2n[m][:, js * 512:(js + 1) * 512],
                                 start=(m == 0), stop=(m == NMI - 1))
            cst = p_sml.tile([1, 512], F32, tag="cs2s", name="cs2s")
            nc.scalar.copy(cst[:], pcs[:])
            nc.sync.dma_start(cs2_in[:, js * 512:(js + 1) * 512], cst[:])
        nc.gpsimd.collective_compute(
            "ReduceScatter", mybir.AluOpType.add, replica_groups=GROUPS,
            ins=[cs2_in.opt()], outs=[cs2_out.opt()])
        deg2 = p_sml.tile([P, NMI], F32, tag="deg2", name="deg2")
        for r2 in range(NMI):
            nc.sync.dma_start(deg2[:, r2:r2 + 1], cs2_out[r2])
        deg2p = p_sml.tile([P, NMI], F32, tag="deg2p", name="deg2p")
        nc.vector.tensor_scalar_add(deg2p[:], deg2[:], 1.0)
        dinv2 = p_sml.tile([P, NMI], F32, tag="dinv2", name="dinv2")
        nc.vector.reciprocal(dinv2[:], deg2p[:])

        # ---- readout partials: P[mt] = sum_kt H2'[kt][:,mt].T @ Xw[kt] --
        for mt in range(NJT):
            pro = ps_msc.tile([P, W_OUT], F32, tag="msc", name="pro")
            for kt in range(NMI):
                nc.tensor.matmul(pro[:], h2n[kt][:, mt * P:(mt + 1) * P],
                                 xw_b[kt][:], start=(kt == 0),
                                 stop=(kt == NMI - 1))
            rot = p_sml.tile([P, W_OUT], F32, tag="ros", name="ros")
            nc.scalar.copy(rot[:], pro[:])
            nc.sync.dma_start(ro_in[mt][:, :], rot[:])
        nc.gpsimd.collective_compute(
            "ReduceScatter", mybir.AluOpType.add, replica_groups=GROUPS,
            ins=[ro_in.opt()], outs=[ro_out.opt()])

        # ---- epilogue: (RS + Xw) * dinv2, relu, store ------------------
        for s in range(NMI):
            t = p_sml.tile([P, W_OUT], F32, tag="ep0", name="ep0")
            nc.sync.dma_start(t[:], ro_out[s])
            t1 = p_sml.tile([P, W_OUT], F32, tag="ep1", name="ep1")
            nc.vector.tensor_add(t1[:], t[:], xw_f[s][:])
            t2 = p_sml.tile([P, W_OUT], F32, tag="ep2", name="ep2")
            nc.vector.tensor_scalar(t2[:], t1[:], dinv2[:, s:s + 1], 0.0,
                                    mybir.AluOpType.mult, mybir.AluOpType.max)
            nc.sync.dma_start(out_x[s * P:(s + 1) * P, :], t2[:])

        for nm in reversed(list(cms)):
            close_pool(nm)

    nc.compile()
    return nc


_NC = None


def _softmax(w):
    w = np.asarray(w, np.float64)
    m = w.max(axis=1, keepdims=True)
    e = np.exp(w - m)
    return (e / e.sum(axis=1, keepdims=True)).astype(np.float32)


def make_in_maps(A, X, conv_w_l0_1, conv_w_l0_2, conv_w_l1, gcn_weight):
    A = np.ascontiguousarray(np.asarray(A, np.float32))
    X = np.asarray(X, np.float32)
    W = np.ascontiguousarray(np.asarray(gcn_weight, np.float32))
    s_a = _softmax(np.asarray(conv_w_l0_1, np.float32)[:, :, 0, 0])
    s_b = _softmax(np.asarray(conv_w_l0_2, np.float32)[:, :, 0, 0])
    s_a1 = _softmax(np.asarray(conv_w_l1, np.float32)[:, :, 0, 0])

    ident = np.eye(P).astype(NPBF)
    idx = np.arange(P)
    in_maps = []
    for k in range(8):
        c, r = k // 4, k % 4
        rows = slice(NB * r, NB * r + NB)
        scal = np.zeros((P, 16), np.float32)
        scal[:, 0:5] = s_a[c]
        scal[:, 5:10] = s_b[c]
        scal[:, 10:15] = s_a1[c]
        # natural mask: {2^-8, 0}, diag at col 512r + 128mi + p
        dmask = np.full((NMI, P, N), NPBF(DESCALE), NPBF)
        for mi in range(NMI):
            dmask[mi, idx, NB * r + P * mi + idx] = NPBF(0.0)
        # transposed mask: {1, 0}, tile jt row p (j=128jt+p), col q=j-512r
        dmaskt = np.ones((NJT, P, NB), NPBF)
        for jt in range(NJT):
            j = P * jt + idx
            q = j - NB * r
            sel = (q >= 0) & (q < NB)
            dmaskt[jt, idx[sel], q[sel]] = NPBF(0.0)
        in_maps.append({
            "a_blk": np.ascontiguousarray(A[:, rows, :]),
            "xt_blk": np.ascontiguousarray(X[rows, :].T),
            "w_gcn": W,
            "scal": scal,
            "ident": ident,
            "dmask": dmask,
            "dmaskt": dmaskt,
        })
    return in_maps


def kernel(A, X, conv_w_l0_1, conv_w_l0_2, conv_w_l1, gcn_weight):
    global _NC
    if _NC is None:
        _NC = _build()
    in_maps = make_in_maps(A, X, conv_w_l0_1, conv_w_l0_2, conv_w_l1,
                           gcn_weight)
    trace = bool(os.environ.get("GTN_TRACE"))
    if trace:
        _register_ntff_hook()
    res = run_bass_kernel_spmd(_NC, in_maps, list(range(8)), trace=trace)
    kernel.last_results = res

    out = np.zeros((N, C * W_OUT), np.float32)
    for k in range(8):
        c, r = k // 4, k % 4
        out[NB * r:NB * r + NB, c * W_OUT:(c + 1) * W_OUT] = \
            res.results[k]["out_x"]
    return out


# revision 18
# speedup vs baseline: 1.9461x; 1.1488x over previous
"""GTN (Graph Transformer Network) message-passing on 8 trn2 NeuronCores.

Problem nn_GTN_17162689314910:
  A: [E=5, N=2048, N] f32, X: [2048, 256] f32, conv_w_*: [C=2, 5, 1, 1] f32,
  gcn_weight: [256, 64] f32 -> out [2048, 128] f32.

Distribution (channel x row split): core k -> channel c=k//4, row block
r=k%4 (512 rows). Per core (heavy math on-device; fp8 wire, f32 accum):
  - combine a,b,a1 gtconv slices from its A rows (DVE; outputs fp8)
  - AllGather b, a1 (fp8) within its 4-core group
  - layer0 computed directly in TRANSPOSED form:
      H'T[jt] = lhsT(b tiles).T @ rhs(a.T tiles), masked by transposed
      diag-mask -> no PE transposes of H at all.
  - colsum partial -> AllReduce -> Hn1T = 256*dinv1 * H'T  (fp8)
  - layer1: H2 = Hn1 @ a1 (fp8 x fp8, x256 scale), drained through the
    natural diag-mask whose values are 2^-8 (mask + descale in one mul)
  - colsum2 partial (PE ones-trick) -> ReduceScatter -> dinv2
  - readout partial H2'.T @ Xw -> ReduceScatter -> +Xw, *dinv2, relu
Host does only: softmax of the 10 conv scalars, sharding/transpose of
input slices, mask/identity constant construction, output concat.
"""
import os
import numpy as np
import ml_dtypes

import concourse.bass as bass
import concourse.tile as tile
from concourse import bacc, mybir
from concourse.bass_utils import run_bass_kernel_spmd
from concourse.tile_rust import add_dep_helper

F32 = mybir.dt.float32
BF16 = mybir.dt.bfloat16
FP8 = mybir.dt.float8e4
NPBF = ml_dtypes.bfloat16
NPF8 = np.dtype(mybir.dt.np(mybir.dt.float8e4))
DESCALE = np.float32(2.0 ** -8)

N, E, C, NB, P = 2048, 5, 2, 512, 128
W_IN, W_OUT = 256, 64
NJT = N // P           # 16 j tiles
NMI = NB // P          # 4 row tiles per core block
GROUPS = [[0, 1, 2, 3], [4, 5, 6, 7]]


def _register_ntff_hook():
    try:
        import antenv.axon_hooks  # noqa: F401
        return
    except ImportError:
        pass
    try:
        import sys, types, antenv
        from trn_agent_boot.trn_boot import _ntff_profile_via_ctypes
        mod = types.ModuleType("antenv.axon_hooks")
        _h = [None]
        mod.set_axon_ntff_profile_hook = lambda h: _h.__setitem__(0, h)
        mod.get_axon_ntff_profile_hook = lambda: _h[0]
        sys.modules["antenv.axon_hooks"] = mod
        antenv.axon_hooks = mod
        mod.set_axon_ntff_profile_hook(
            _ntff_profile_via_ctypes("/opt/axon/libaxon_pjrt.so"))
    except Exception:
        pass


def _build():
    nc = bacc.Bacc("TRN2", target_bir_lowering=False, debug=False,
                   num_devices=8)

    a_blk = nc.dram_tensor("a_blk", [E, NB, N], BF16, kind="ExternalInput").ap()
    xt_blk = nc.dram_tensor("xt_blk", [W_IN, NB], F32, kind="ExternalInput").ap()
    w_gcn = nc.dram_tensor("w_gcn", [W_IN, W_OUT], F32, kind="ExternalInput").ap()
    scal = nc.dram_tensor("scal", [P, 16], F32, kind="ExternalInput").ap()
    ident = nc.dram_tensor("ident", [P, P], BF16, kind="ExternalInput").ap()
    # natural diag mask, values {2^-8, 0} (mask + layer-1 descale in one mul)
    dmask = nc.dram_tensor("dmask", [NMI, P, N], BF16, kind="ExternalInput").ap()
    # transposed diag mask, values {1, 0}
    dmaskt = nc.dram_tensor("dmaskt", [NJT, P, NB], BF16,
                            kind="ExternalInput").ap()
    out_x = nc.dram_tensor("out_x", [NB, W_OUT], F32, kind="ExternalOutput").ap()

    with tile.TileContext(nc) as tc:
        cms = {}

        def open_pool(**kw):
            cm = tc.tile_pool(**kw)
            pool = cm.__enter__()
            cms[kw["name"]] = cm
            return pool

        def close_pool(name):
            cms.pop(name).__exit__(None, None, None)

        dram = open_pool(name="dram", bufs=1, space="DRAM")
        const = open_pool(name="const", bufs=1)
        ps_mm = open_pool(name="ps_mm", bufs=4, space="PSUM")
        ps_tr = open_pool(name="ps_tr", bufs=2, space="PSUM")
        ps_msc = open_pool(name="ps_msc", bufs=2, space="PSUM")
        p_agt = open_pool(name="p_agt", bufs=NJT)
        p_mskt = open_pool(name="p_mskt", bufs=NJT)
        p_hnt = open_pool(name="p_hnt", bufs=NJT)
        p_hn1 = open_pool(name="p_hn1", bufs=NJT)
        p_sml = open_pool(name="p_sml", bufs=2)

        # dram comm buffers (fp8 wire for the big gathers)
        ag_b_in = dram.tile([NB, N], FP8, tag="agbi", name="agbi")
        ag_b_out = dram.tile([N, N], FP8, tag="agbo", name="agbo")
        ag_c_in = dram.tile([NB, N], FP8, tag="agci", name="agci")
        ag_c_out = dram.tile([N, N], FP8, tag="agco", name="agco")
        cs1_in = dram.tile([P, NJT], F32, tag="cs1i", name="cs1i")
        cs1_out = dram.tile([P, NJT], F32, tag="cs1o", name="cs1o")
        cs2_in = dram.tile([1, N], F32, tag="cs2i", name="cs2i")
        cs2_out = dram.tile([NMI, P, 1], F32, tag="cs2o", name="cs2o")
        ro_in = dram.tile([NJT, P, W_OUT], F32, tag="roi", name="roi")
        ro_out = dram.tile([NMI, P, W_OUT], F32, tag="roo", name="roo")

        # constants
        sc = const.tile([P, 16], F32, tag="sc", name="sc")
        nc.sync.dma_start(sc[:], scal[:])
        idt = const.tile([P, P], BF16, tag="idt", name="idt")
        nc.sync.dma_start(idt[:], ident[:])
        ones = const.tile([P, 1], BF16, tag="ones", name="ones")
        nc.vector.memset(ones[:], 1.0)
        mskt = []
        for jt in range(NJT):
            mk = p_mskt.tile([P, NB], BF16, tag="mskt", name="mskt")
            nc.sync.dma_start(mk[:], dmaskt[jt])
            mskt.append(mk)

        # ---- Xw = X[rows] @ W  (f32, tiny) ------------------------------
        xtt = [const.tile([P, NB], F32, tag=f"xtt{d}", name=f"xtt{d}")
               for d in range(2)]
        wt = [const.tile([P, W_OUT], F32, tag=f"wt{d}", name=f"wt{d}")
              for d in range(2)]
        for d in range(2):
            nc.sync.dma_start(xtt[d][:], xt_blk[d * P:(d + 1) * P, :])
            nc.sync.dma_start(wt[d][:], w_gcn[d * P:(d + 1) * P, :])
        xw_f = [const.tile([P, W_OUT], F32, tag=f"xwf{m}", name=f"xwf{m}")
                for m in range(NMI)]
        xw_b = [const.tile([P, W_OUT], BF16, tag=f"xwb{m}", name=f"xwb{m}")
                for m in range(NMI)]
        for m in range(NMI):
            pxw = ps_msc.tile([P, W_OUT], F32, tag="msc", name="pxw")
            for d in range(2):
                nc.tensor.matmul(pxw[:], xtt[d][:, m * P:(m + 1) * P], wt[d][:],
                                 start=(d == 0), stop=(d == 1))
            nc.scalar.copy(xw_f[m][:], pxw[:])
            nc.vector.tensor_copy(xw_b[m][:], pxw[:])

        # ---- prologue: stage A (cast to bf16), combines, AGs ------------
        # b-combines first so AG(b) launches early and overlaps the rest.
        p_anat = open_pool(name="p_anat", bufs=NMI)
        p_nat = open_pool(name="p_nat", bufs=2)
        astage = open_pool(name="astage", bufs=E * NMI)

        at_all = []
        for mi in range(NMI):
            at = [astage.tile([P, N], BF16, tag="ast", name="ast")
                  for _ in range(E)]
            for e in range(E):
                nc.sync.dma_start(at[e][:], a_blk[e, mi * P:(mi + 1) * P, :])
            at_all.append(at)

        def combine(at, dst, col0):
            # scales split DVE(3)/ACT(2); adds on DVE
            t0 = p_nat.tile([P, N], BF16, tag="ct0", name="ct0")
            t1 = p_nat.tile([P, N], BF16, tag="ct1", name="ct1")
            t2 = p_nat.tile([P, N], BF16, tag="ct2", name="ct2")
            t3 = p_nat.tile([P, N], BF16, tag="ct3", name="ct3")
            nc.vector.tensor_scalar_mul(t0[:], at[0][:], sc[:, col0:col0 + 1])
            nc.vector.tensor_scalar_mul(t1[:], at[1][:],
                                        sc[:, col0 + 1:col0 + 2])
            nc.scalar.mul(t2[:], at[2][:], sc[:, col0 + 2:col0 + 3])
            nc.scalar.mul(t3[:], at[3][:], sc[:, col0 + 3:col0 + 4])
            nc.vector.tensor_add(t0[:], t0[:], t1[:])
            nc.vector.tensor_scalar_mul(t1[:], at[4][:],
                                        sc[:, col0 + 4:col0 + 5])
            nc.vector.tensor_add(t2[:], t2[:], t3[:])
            nc.vector.tensor_add(t0[:], t0[:], t2[:])
            nc.vector.tensor_add(dst[:], t0[:], t1[:])

        for mi in range(NMI):
            b_nat = p_nat.tile([P, N], FP8, tag="bnat", name="bnat")
            combine(at_all[mi], b_nat, 5)
            nc.sync.dma_start(ag_b_in[mi * P:(mi + 1) * P, :], b_nat[:])

        nc.gpsimd.collective_compute(
            "AllGather", mybir.AluOpType.bypass, replica_groups=GROUPS,
            ins=[ag_b_in.opt()], outs=[ag_b_out.opt()])

        a_nat = []
        for mi in range(NMI):
            an = p_anat.tile([P, N], BF16, tag="anat", name="anat")
            combine(at_all[mi], an, 0)
            a_nat.append(an)
            c_nat = p_nat.tile([P, N], FP8, tag="cnat", name="cnat")
            combine(at_all[mi], c_nat, 10)
            nc.sync.dma_start(ag_c_in[mi * P:(mi + 1) * P, :], c_nat[:])

        ag_c_cc = nc.gpsimd.collective_compute(
            "AllGather", mybir.AluOpType.bypass, replica_groups=GROUPS,
            ins=[ag_c_in.opt()], outs=[ag_c_out.opt()])

        close_pool("astage")
        close_pool("p_nat")

        # transpose a_nat -> a_gT tiles [128(j), 512(i)] fp8 (PE transpose)
        agt = [p_agt.tile([P, NB], FP8, tag="agt", name="agt")
               for _ in range(NJT)]
        for mi in range(NMI):
            for jt in range(NJT):
                pt = ps_tr.tile([P, P], BF16, tag="tp", name="tp")
                nc.tensor.transpose(pt[:], a_nat[mi][:, jt * P:(jt + 1) * P],
                                    idt[:])
                nc.scalar.copy(agt[jt][:, mi * P:(mi + 1) * P], pt[:])

        close_pool("p_anat")

        p_msk = open_pool(name="p_msk", bufs=NJT)
        msk = [[None] * 4 for _ in range(NMI)]
        for m in range(NMI):
            for nq in range(4):
                mk = p_msk.tile([P, 512], BF16, tag="msk", name="msk")
                nc.sync.dma_start(mk[:], dmask[m, :, nq * 512:(nq + 1) * 512])
                msk[m][nq] = mk
        p_h2n = open_pool(name="p_h2n", bufs=NMI)
        # b tiles resident (fp8)
        p_bsb = open_pool(name="p_bsb", bufs=NJT)
        b_sb = []
        for jt in range(NJT):
            t = p_bsb.tile([P, N], FP8, tag="bsb", name="bsb")
            bd = nc.sync.dma_start(t[:], ag_b_out[jt * P:(jt + 1) * P, :])
            add_dep_helper(ag_c_cc.ins, bd.ins, True,
                           "AG(a1) after b_sb loads (SDMA contention)")
            b_sb.append(t)

        # ---- layer 0 in transposed form --------------------------------
        # H'T[jt][p, i] = sum_k b[k, 128jt+p] * a[own_i, k], masked.
        # lhsT = b_sb[kt][:, jt*128:+128]  (K=128 rows of b, M=128 j-cols)
        # rhs  = agt[kt]                   (K=128, N=512 own rows)
        cs1_sb = p_sml.tile([P, NJT], F32, tag="cs1s", name="cs1s")
        hnt = []
        for jt in range(NJT):
            pq = ps_mm.tile([P, NB], F32, tag="hacc", name="hacc")
            for kt in range(NJT):
                nc.tensor.matmul(pq[:], b_sb[kt][:, jt * P:(jt + 1) * P],
                                 agt[kt][:], start=(kt == 0),
                                 stop=(kt == NJT - 1))
            ht = p_hnt.tile([P, NB], BF16, tag="hnt", name="hnt")
            nc.vector.tensor_mul(ht[:], pq[:], mskt[jt][:])
            nc.vector.tensor_reduce(cs1_sb[:, jt:jt + 1], ht[:],
                                    mybir.AxisListType.X, mybir.AluOpType.add)
            hnt.append(ht)

        nc.sync.dma_start(cs1_in[:], cs1_sb[:])
        nc.gpsimd.collective_compute(
            "AllReduce", mybir.AluOpType.add, replica_groups=GROUPS,
            ins=[cs1_in.opt()], outs=[cs1_out.opt()])
        deg1 = p_sml.tile([P, NJT], F32, tag="deg1", name="deg1")
        nc.sync.dma_start(deg1[:], cs1_out[:])
        dinv1 = p_sml.tile([P, NJT], F32, tag="dinv1", name="dinv1")
        nc.vector.reciprocal(dinv1[:], deg1[:])
        d256 = p_sml.tile([P, NJT], F32, tag="d256", name="d256")
        nc.vector.tensor_scalar_mul(d256[:], dinv1[:], 256.0)
        # Hn1T (fp8, x256 scaled)
        hn1 = []
        for jt in range(NJT):
            h8 = p_hn1.tile([P, NB], FP8, tag="hn1", name="hn1")
            nc.scalar.mul(h8[:], hnt[jt][:], d256[:, jt:jt + 1])
            hn1.append(h8)

        # a1 tiles resident (fp8)
        p_csb = open_pool(name="p_csb", bufs=NJT)
        c_sb = []
        for jt in range(NJT):
            t = p_csb.tile([P, N], FP8, tag="csb", name="csb")
            nc.sync.dma_start(t[:], ag_c_out[jt * P:(jt + 1) * P, :])
            c_sb.append(t)

        # ---- layer 1: H2 = Hn1 @ a1 (fp8, x256), drain descales --------
        h2n = [p_h2n.tile([P, N], BF16, tag="h2n", name="h2n")
               for _ in range(NMI)]
        for nq in range(4):
            pq = [ps_mm.tile([P, 512], F32, tag="hacc", name="hacc1")
                  for _ in range(NMI)]
            for kt in range(NJT):
                for m in range(NMI):
                    nc.tensor.matmul(
                        pq[m][:], hn1[kt][:, m * P:(m + 1) * P],
                        c_sb[kt][:, nq * 512:(nq + 1) * 512],
                        start=(kt == 0), stop=(kt == NJT - 1))
            for m in range(NMI):
                nc.vector.tensor_mul(h2n[m][:, nq * 512:(nq + 1) * 512],
                                     pq[m][:], msk[m][nq][:])

        # ---- colsum2 (PE ones-trick) -> RS -> dinv2 --------------------
        for js in range(4):
            pcs = ps_msc.tile([1, 512], F32, tag="msc", name="pcs")
            for m in range(NMI):
                nc.tensor.matmul(pcs[:], ones[:],
                                 h2n[m][:, js * 512:(js + 1) * 512],
                                 start=(m == 0), stop=(m == NMI - 1))
            cst = p_sml.tile([1, 512], F32, tag="cs2s", name="cs2s")
            nc.scalar.copy(cst[:], pcs[:])
            nc.sync.dma_start(cs2_in[:, js * 512:(js + 1) * 512], cst[:])
        nc.gpsimd.collective_compute(
            "ReduceScatter", mybir.AluOpType.add, replica_groups=GROUPS,
            ins=[cs2_in.opt()], outs=[cs2_out.opt()])
        deg2 = p_sml.tile([P, NMI], F32, tag="deg2", name="deg2")
        for r2 in range(NMI):
            nc.sync.dma_start(deg2[:, r2:r2 + 1], cs2_out[r2])
        deg2p = p_sml.tile([P, NMI], F32, tag="deg2p", name="deg2p")
        nc.vector.tensor_scalar_add(deg2p[:], deg2[:], 1.0)
        dinv2 = p_sml.tile([P, NMI], F32, tag="dinv2", name="dinv2")
        nc.vector.reciprocal(dinv2[:], deg2p[:])

        # ---- readout partials: P[mt] = sum_kt H2'[kt][:,mt].T @ Xw[kt] --
        for mt in range(NJT):
            pro = ps_msc.tile([P, W_OUT], F32, tag="msc", name="pro")
            for kt in range(NMI):
                nc.tensor.matmul(pro[:], h2n[kt][:, mt * P:(mt + 1) * P],
                                 xw_b[kt][:], start=(kt == 0),
                                 stop=(kt == NMI - 1))
            rot = p_sml.tile([P, W_OUT], F32, tag="ros", name="ros")
            nc.scalar.copy(rot[:], pro[:])
            nc.sync.dma_start(ro_in[mt][:, :], rot[:])
        nc.gpsimd.collective_compute(
            "ReduceScatter", mybir.AluOpType.add, replica_groups=GROUPS,
            ins=[ro_in.opt()], outs=[ro_out.opt()])

        # ---- epilogue: (RS + Xw) * dinv2, relu, store ------------------
        for s in range(NMI):
            t = p_sml.tile([P, W_OUT], F32, tag="ep0", name="ep0")
            nc.sync.dma_start(t[:], ro_out[s])
            t1 = p_sml.tile([P, W_OUT], F32, tag="ep1", name="ep1")
            nc.vector.tensor_add(t1[:], t[:], xw_f[s][:])
            t2 = p_sml.tile([P, W_OUT], F32, tag="ep2", name="ep2")
            nc.vector.tensor_scalar(t2[:], t1[:], dinv2[:, s:s + 1], 0.0,
                                    mybir.AluOpType.mult, mybir.AluOpType.max)
            nc.sync.dma_start(out_x[s * P:(s + 1) * P, :], t2[:])

        for nm in reversed(list(cms)):
            close_pool(nm)

    nc.compile()
    return nc


_NC = None


def _softmax(w):
    w = np.asarray(w, np.float64)
    m = w.max(axis=1, keepdims=True)
    e = np.exp(w - m)
    return (e / e.sum(axis=1, keepdims=True)).astype(np.float32)


def make_in_maps(A, X, conv_w_l0_1, conv_w_l0_2, conv_w_l1, gcn_weight):
    A = np.ascontiguousarray(np.asarray(A, np.float32))
    X = np.asarray(X, np.float32)
    W = np.ascontiguousarray(np.asarray(gcn_weight, np.float32))
    s_a = _softmax(np.asarray(conv_w_l0_1, np.float32)[:, :, 0, 0])
    s_b = _softmax(np.asarray(conv_w_l0_2, np.float32)[:, :, 0, 0])
    s_a1 = _softmax(np.asarray(conv_w_l1, np.float32)[:, :, 0, 0])

    ident = np.eye(P).astype(NPBF)
    idx = np.arange(P)
    in_maps = []
    for k in range(8):
        c, r = k // 4, k % 4
        rows = slice(NB * r, NB * r + NB)
        scal = np.zeros((P, 16), np.float32)
        scal[:, 0:5] = s_a[c]
        scal[:, 5:10] = s_b[c]
        scal[:, 10:15] = s_a1[c]
        # natural mask: {2^-8, 0}, diag at col 512r + 128mi + p
        dmask = np.full((NMI, P, N), NPBF(DESCALE), NPBF)
        for mi in range(NMI):
            dmask[mi, idx, NB * r + P * mi + idx] = NPBF(0.0)
        # transposed mask: {1, 0}, tile jt row p (j=128jt+p), col q=j-512r
        dmaskt = np.ones((NJT, P, NB), NPBF)
        for jt in range(NJT):
            j = P * jt + idx
            q = j - NB * r
            sel = (q >= 0) & (q < NB)
            dmaskt[jt, idx[sel], q[sel]] = NPBF(0.0)
        in_maps.append({
            "a_blk": np.ascontiguousarray(A[:, rows, :]).astype(NPBF),
            "xt_blk": np.ascontiguousarray(X[rows, :].T),
            "w_gcn": W,
            "scal": scal,
            "ident": ident,
            "dmask": dmask,
            "dmaskt": dmaskt,
        })
    return in_maps


def kernel(A, X, conv_w_l0_1, conv_w_l0_2, conv_w_l1, gcn_weight):
    global _NC
    if _NC is None:
        _NC = _build()
    in_maps = make_in_maps(A, X, conv_w_l0_1, conv_w_l0_2, conv_w_l1,
                           gcn_weight)
    trace = bool(os.environ.get("GTN_TRACE"))
    if trace:
        _register_ntff_hook()
    res = run_bass_kernel_spmd(_NC, in_maps, list(range(8)), trace=trace)
    kernel.last_results = res

    out = np.zeros((N, C * W_OUT), np.float32)
    for k in range(8):
        c, r = k // 4, k % 4
        out[NB * r:NB * r + NB, c * W_OUT:(c + 1) * W_OUT] = \
            res.results[k]["out_x"]
    return out
